# revision 1
# baseline (speedup 1.0000x reference)
"""Criss-cross attention (CC module) Trainium2 Bass kernel.

Shapes (full): x2,x1 [8, 512, 64, 64] fp32; q_w,k_w [64, 512]; v_w [512, 512];
biases; gamma [1]. Outputs (y2, y1) same shape as x2/x1.

Distribution: data-parallel over batch B=8 across the 8 NeuronCores, one batch
element per core. All sharding/packing and the output un-permutation happen on
host; the device program is a single-core SPMD NEFF.

Per-core algorithm (C=512, CQ=64, H=W=64, S=4096):
  q,k = qk_w^T @ x2 (+bias)                      [128, S]   (packed q|k)
  E_H^T[H',(w,h)] = k[:,:,w]^T q[:,:,w]  per w   (key-on-partition layout)
  E_W^T[W',(h,w)] = k[:,h,:]^T q[:,h,:]  per h
  att = exp(E) (no max shift; |E|max ~ 63 verified), normalized by
  gamma/sum via ones-matmul sums + reciprocal + K=1 broadcast matmul.
  vT[s, co] = x^T v_w^T computed directly by operand-swapped GEMM.
  Pass 1 (rows s = h*64+w): out_W^T psum = sum_h-pair att_W^T-MMs
     + identity-inject of (x^T + gamma*v_b) -> partial (bf16).
  vT chunks stored to DRAM scratch with rows permuted to s' = w*64+h.
  Partial rearranged h-major -> w-major via SBUF->SBUF DMA.
  Pass 2 (rows s' = w*64+h): out_H^T psum + identity-inject(partial)
     -> y^T bf16 -> gpsimd cast-DMA to fp32 DRAM (w-major rows).
  Host: y[c,h,w] = yT[w*64+h, c].

The v-bias is folded through the softmax identity sum(att_H)+sum(att_W)=1:
y = x + gamma*(out_H + out_W + v_b) with att pre-scaled by gamma and
(x^T + gamma*v_b) injected once in pass 1.
"""

import numpy as np
import ml_dtypes

import concourse.bass as bass
import concourse.mybir as mybir
import concourse.tile as tile
from concourse import bacc
from concourse.bass_utils import run_bass_kernel_spmd
from concourse.masks import make_identity

BF16 = mybir.dt.bfloat16
F32 = mybir.dt.float32

B, C, H, W = 8, 512, 64, 64
CQ = 64
S = H * W  # 4096
NCH = S // 128  # 32 spatial chunks of 128 rows
KC = C // 128  # 4 contraction chunks
NB = S // 512  # 8 column blocks of 512

_CACHED = {}


def build_nc(gamma: float):
    nc = bacc.Bacc("TRN2", target_bir_lowering=False, debug=False)

    # Per-core external inputs
    x2f = nc.dram_tensor("x2f", [NB, KC, 128, 512], F32, kind="ExternalInput")
    x2b = nc.dram_tensor("x2b", [NCH, KC, 128, 128], BF16, kind="ExternalInput")
    x1b = nc.dram_tensor("x1b", [NCH, KC, 128, 128], BF16, kind="ExternalInput")
    xt2 = nc.dram_tensor("xt2", [S, C], BF16, kind="ExternalInput")
    xt1 = nc.dram_tensor("xt1", [S, C], BF16, kind="ExternalInput")
    qkw = nc.dram_tensor("qkw", [C, 128], F32, kind="ExternalInput")
    qkb = nc.dram_tensor("qkb", [1, 128], F32, kind="ExternalInput")
    vwtb = nc.dram_tensor("vwtb", [C, C], BF16, kind="ExternalInput")

    y2t = nc.dram_tensor("y2t", [S, C], F32, kind="ExternalOutput")
    y1t = nc.dram_tensor("y1t", [S, C], F32, kind="ExternalOutput")

    # Internal DRAM scratch for the w-major-permuted vT tensors
    vt2_scr = nc.dram_tensor("vt2_scr", [S, C], BF16)
    vt1_scr = nc.dram_tensor("vt1_scr", [S, C], BF16)

    with tile.TileContext(nc) as tc:
        with (
            tc.tile_pool(name="persist", bufs=1) as pp,
            tc.tile_pool(name="ring", bufs=3) as ring,
            tc.tile_pool(name="psum", bufs=8, space="PSUM") as psp,
        ):
            # ---- persistent tiles ----
            qkw_t = [pp.tile([128, 128], F32, tag=f"qkw_{i}", name=f"qkw_{i}") for i in range(KC)]
            vwtb_t = [pp.tile([128, C], BF16, tag=f"vwtb_{i}", name=f"vwtb_{i}") for i in range(KC)]
            qkb_t = pp.tile([1, 128], F32, tag="qkb", name="qkb")
            q_t = pp.tile([64, S], F32, tag="q_t", name="q_t")
            k_t = pp.tile([64, S], F32, tag="k_t", name="k_t")
            # att1 = gamma-scaled exp(E_H^T) [H', (w,h)], att2 = exp(E_W^T) [W', (h,w)]
            # rows 64:128 duplicate rows 0:64 (base-partition alignment for matmul)
            att1 = pp.tile([128, S], BF16, tag="att1", name="att1")
            att2 = pp.tile([128, S], BF16, tag="att2", name="att2")
            ones_row = pp.tile([1, 512], F32, tag="ones_row", name="ones_row")
            ones64 = pp.tile([64, 1], BF16, tag="ones64", name="ones64")
            gam_row = pp.tile([1, 128], BF16, tag="gam_row", name="gam_row")
            r_row = pp.tile([1, S], BF16, tag="r_row", name="r_row")  # 1/sum, (w,h) order
            ident = pp.tile([128, 128], BF16, tag="ident", name="ident")
            part2 = pp.tile([128, NCH * 512], BF16, tag="part2", name="part2")
            part1 = pp.tile([128, NCH * 512], BF16, tag="part1", name="part1")

            nc.gpsimd.memset(ones_row[:], 1.0)
            nc.gpsimd.memset(ones64[:], 1.0)
            nc.gpsimd.memset(gam_row[:], gamma)
            make_identity(nc, ident[:])

            for i in range(KC):
                nc.sync.dma_start(qkw_t[i][:], qkw[128 * i : 128 * (i + 1), :])
                nc.sync.dma_start(vwtb_t[i][:], vwtb[128 * i : 128 * (i + 1), :])
            nc.sync.dma_start(qkb_t[:], qkb[:])

            # ---- Phase B: q,k projection ----
            for n in range(NB):
                cols = slice(512 * n, 512 * (n + 1))
                x2c = ring.tile([128, KC * 512], F32, tag="x2c", bufs=2, name="x2c")
                for kc in range(KC):
                    nc.sync.dma_start(
                        x2c[:, 512 * kc : 512 * (kc + 1)], x2f[n, kc, :, :]
                    )
                psq = psp.tile([64, 512], F32, tag="ps", name="ps_q")
                psk = psp.tile([64, 512], F32, tag="ps", name="ps_k")
                for kc in range(KC):
                    nc.tensor.matmul(
                        psq[:],
                        lhsT=qkw_t[kc][:, 0:64],
                        rhs=x2c[:, 512 * kc : 512 * (kc + 1)],
                        start=(kc == 0),
                        stop=False,
                    )
                    nc.tensor.matmul(
                        psk[:],
                        lhsT=qkw_t[kc][:, 64:128],
                        rhs=x2c[:, 512 * kc : 512 * (kc + 1)],
                        start=(kc == 0),
                        stop=False,
                    )
                nc.tensor.matmul(
                    psq[:], lhsT=qkb_t[0:1, 0:64], rhs=ones_row[:],
                    start=False, stop=True,
                )
                nc.tensor.matmul(
                    psk[:], lhsT=qkb_t[0:1, 64:128], rhs=ones_row[:],
                    start=False, stop=True,
                )
                nc.scalar.activation(
                    out=q_t[:, cols], in_=psq[:],
                    func=mybir.ActivationFunctionType.Copy,
                )
                nc.scalar.activation(
                    out=k_t[:, cols], in_=psk[:],
                    func=mybir.ActivationFunctionType.Copy,
                )

            qv = q_t[:].rearrange("p (h w) -> p h w", w=W)
            kv = k_t[:].rearrange("p (h w) -> p h w", w=W)
            # h = 2m + e decomposition: att1 columns stored (w-block | e, m)
            # so pass-2 stationary slices are contiguous single-free-dim APs
            qv_em = q_t[:].rearrange("p (m e w) -> p e m w", e=2, w=W)

            # ---- Phase C: energies (transposed) + exp ----
            for b8 in range(NB):
                psh = psp.tile([128, 512], F32, tag="ps", name="ps_eh")
                psw = psp.tile([128, 512], F32, tag="ps", name="ps_ew")
                for i in range(8):
                    w = 8 * b8 + i
                    for half in range(2):
                        rows = slice(64 * half, 64 * (half + 1))
                        nc.tensor.matmul(
                            psh[rows, 64 * i : 64 * (i + 1)],
                            lhsT=kv[:, :, w],
                            rhs=qv_em[:, :, :, w],
                            start=True,
                            stop=True,
                        )
                        nc.tensor.matmul(
                            psw[rows, 64 * i : 64 * (i + 1)],
                            lhsT=kv[:, w, :],
                            rhs=qv[:, w, :],
                            start=True,
                            stop=True,
                        )
                nc.scalar.activation(
                    out=att1[:, 512 * b8 : 512 * (b8 + 1)],
                    in_=psh[:],
                    func=mybir.ActivationFunctionType.Exp,
                )
                nc.scalar.activation(
                    out=att2[:, 512 * b8 : 512 * (b8 + 1)],
                    in_=psw[:],
                    func=mybir.ActivationFunctionType.Exp,
                )

            # ---- Phase D: sums over the 128 keys of each query ----
            # (w,h)-ordered query index. att2 read through a reordering AP.
            att2_v = att2[0:64].rearrange("p (m e w) -> p w e m", e=2, w=W)
            for n in range(NB):
                ps = psp.tile([1, 512], F32, tag="ps", name="ps_s")
                nc.tensor.matmul(
                    ps[:],
                    lhsT=ones64[:],
                    rhs=att1[0:64, 512 * n : 512 * (n + 1)],
                    start=True,
                    stop=False,
                )
                nc.tensor.matmul(
                    ps[:],
                    lhsT=ones64[:],
                    rhs=att2_v[:, 8 * n : 8 * (n + 1), :, :],
                    start=False,
                    stop=True,
                )
                with nc.allow_low_precision(reason="softmax recip row in bf16"):
                    nc.vector.reciprocal(
                        r_row[0:1, 512 * n : 512 * (n + 1)], ps[:]
                    )

            # ---- Phase E: broadcast gamma/s and normalize att in place ----
            # r_row columns are (w, e, m); att2 needs (h=(m,e), w) order
            r_em = r_row[0:1].rearrange("p (w e m) -> p m e w", e=2, m=32)
            for n in range(NB):
                cols = slice(512 * n, 512 * (n + 1))
                ps = psp.tile([128, 512], F32, tag="ps", name="ps_r")
                nc.tensor.matmul(
                    ps[:],
                    lhsT=gam_row[:],
                    rhs=r_row[0:1, cols],
                    start=True,
                    stop=True,
                )
                nc.vector.tensor_mul(att1[:, cols], att1[:, cols], ps[:])
                ps2 = psp.tile([128, 512], F32, tag="ps", name="ps_r")
                nc.tensor.matmul(
                    ps2[:],
                    lhsT=gam_row[:],
                    rhs=r_em[:, 4 * n : 4 * (n + 1), :, :],
                    start=True,
                    stop=True,
                )
                nc.vector.tensor_mul(att2[:, cols], att2[:, cols], ps2[:])

            # Scratch views for permuted writes: row w*64+h <- produced (h,w)
            vt2_v = vt2_scr.rearrange("(w h) c -> h w c", h=H)
            vt1_v = vt1_scr.rearrange("(w h) c -> h w c", h=H)

            # ---- Phase F: pass 1 (W-attention, h-major chunks) ----
            for j in range(NCH):
                sl = slice(128 * j, 128 * (j + 1))
                # vT chunks via operand-swapped GEMM
                x2cc = ring.tile([128, KC * 128], BF16, tag="x2cc", name="x2cc")
                for kc in range(KC):
                    nc.sync.dma_start(
                        x2cc[:, 128 * kc : 128 * (kc + 1)], x2b[j, kc, :, :]
                    )
                psv2 = psp.tile([128, C], F32, tag="ps", name="ps_v2")
                for kc in range(KC):
                    nc.tensor.matmul(
                        psv2[:],
                        lhsT=x2cc[:, 128 * kc : 128 * (kc + 1)],
                        rhs=vwtb_t[kc][:],
                        start=(kc == 0),
                        stop=(kc == KC - 1),
                    )
                v2t = ring.tile([128, C], BF16, tag="v2t", name="v2t")
                nc.scalar.activation(
                    out=v2t[:], in_=psv2[:], func=mybir.ActivationFunctionType.Copy
                )
                nc.sync.dma_start(vt2_v[2 * j : 2 * j + 2, :, :], v2t[:])

                x1c = ring.tile([128, KC * 128], BF16, tag="x1c", name="x1c")
                for kc in range(KC):
                    nc.sync.dma_start(
                        x1c[:, 128 * kc : 128 * (kc + 1)], x1b[j, kc, :, :]
                    )
                psv1 = psp.tile([128, C], F32, tag="ps", name="ps_v1")
                for kc in range(KC):
                    nc.tensor.matmul(
                        psv1[:],
                        lhsT=x1c[:, 128 * kc : 128 * (kc + 1)],
                        rhs=vwtb_t[kc][:],
                        start=(kc == 0),
                        stop=(kc == KC - 1),
                    )
                v1t = ring.tile([128, C], BF16, tag="v1t", name="v1t")
                nc.scalar.activation(
                    out=v1t[:], in_=psv1[:], func=mybir.ActivationFunctionType.Copy
                )
                nc.sync.dma_start(vt1_v[2 * j : 2 * j + 2, :, :], v1t[:])

                xt2c = ring.tile([128, C], BF16, tag="xt2c", name="xt2c")
                nc.sync.dma_start(xt2c[:], xt2[sl, :])
                xt1c = ring.tile([128, C], BF16, tag="xt1c", name="xt1c")
                nc.sync.dma_start(xt1c[:], xt1[sl, :])

                pso2 = psp.tile([128, C], F32, tag="ps", name="ps_o2")
                pso1 = psp.tile([128, C], F32, tag="ps", name="ps_o1")
                nc.tensor.matmul(
                    pso2[:], lhsT=ident[:], rhs=xt2c[:], start=True, stop=False,
                    skip_group_check=True,
                )
                nc.tensor.matmul(
                    pso1[:], lhsT=ident[:], rhs=xt1c[:], start=True, stop=False,
                    skip_group_check=True,
                )
                for hp in range(2):
                    h = 2 * j + hp
                    rows = slice(64 * hp, 64 * (hp + 1))
                    att_w = att2[rows, 64 * h : 64 * (h + 1)]
                    nc.tensor.matmul(
                        pso2[rows, :], lhsT=att_w, rhs=v2t[rows, :],
                        start=False, stop=True, skip_group_check=True,
                    )
                    nc.tensor.matmul(
                        pso1[rows, :], lhsT=att_w, rhs=v1t[rows, :],
                        start=False, stop=True, skip_group_check=True,
                    )
                nc.scalar.activation(
                    out=part2[:, 512 * j : 512 * (j + 1)],
                    in_=pso2[:],
                    func=mybir.ActivationFunctionType.Copy,
                )
                nc.scalar.activation(
                    out=part1[:, 512 * j : 512 * (j + 1)],
                    in_=pso1[:],
                    func=mybir.ActivationFunctionType.Copy,
                )

            # Views of partials for the h-major -> w-major rearrange.
            # partial element for spatial s=(h*64+w) lives at
            # partition (s%128), free (s//128)*512 + c.
            # partial source view: partition (e*64 + w), free (m, c);
            # element = partial for query (h=2m+e, w) of spatial s=h*64+w
            p2v = part2.rearrange("(e p) (m c) -> e p m c", e=2, c=512)
            p1v = part1.rearrange("(e p) (m c) -> e p m c", e=2, c=512)

            # ---- Phase G/H: pass 2 (H-attention, w-major chunks) ----
            for j in range(NCH):
                sl = slice(128 * j, 128 * (j + 1))
                v2t = ring.tile([128, C], BF16, tag="v2tw", name="v2tw")
                nc.sync.dma_start(v2t[:], vt2_scr[sl, :])
                v1t = ring.tile([128, C], BF16, tag="v1tw", name="v1tw")
                nc.sync.dma_start(v1t[:], vt1_scr[sl, :])

                # rearranged partial chunk: rows r'=wp*64+2m+e hold s=h*64+w,
                # w=2j+wp, h=2m+e
                # p2c rows r = wp*64 + e*32 + m hold partial of query
                # (h=2m+e, w=2j+wp); each (wp,e) DMA writes a contiguous
                # 32-row block from one source partition of part*.
                p2c = ring.tile([128, C], BF16, tag="p2c", name="p2c")
                p1c = ring.tile([128, C], BF16, tag="p1c", name="p1c")
                for wp in range(2):
                    for e in range(2):
                        w = 2 * j + wp
                        rows = slice(64 * wp + 32 * e, 64 * wp + 32 * (e + 1))
                        nc.sync.dma_start(
                            p2c[rows, :], p2v[e, w : w + 1, :, :]
                        )
                        nc.sync.dma_start(
                            p1c[rows, :], p1v[e, w : w + 1, :, :]
                        )

                psf2 = psp.tile([128, C], F32, tag="ps", name="ps_f2")
                psf1 = psp.tile([128, C], F32, tag="ps", name="ps_f1")
                nc.tensor.matmul(
                    psf2[:], lhsT=ident[:], rhs=p2c[:], start=True, stop=False,
                    skip_group_check=True,
                )
                nc.tensor.matmul(
                    psf1[:], lhsT=ident[:], rhs=p1c[:], start=True, stop=False,
                    skip_group_check=True,
                )
                for wp in range(2):
                    w = 2 * j + wp
                    rows = slice(64 * wp, 64 * (wp + 1))
                    # columns already stored (e, m): h = 2m + e
                    att_h = att1[rows, 64 * w : 64 * (w + 1)]
                    nc.tensor.matmul(
                        psf2[rows, :], lhsT=att_h, rhs=v2t[rows, :],
                        start=False, stop=True, skip_group_check=True,
                    )
                    nc.tensor.matmul(
                        psf1[rows, :], lhsT=att_h, rhs=v1t[rows, :],
                        start=False, stop=True, skip_group_check=True,
                    )
                y2c = ring.tile([128, C], BF16, tag="y2c", name="y2c")
                nc.scalar.activation(
                    out=y2c[:], in_=psf2[:], func=mybir.ActivationFunctionType.Copy
                )
                y1c = ring.tile([128, C], BF16, tag="y1c", name="y1c")
                nc.scalar.activation(
                    out=y1c[:], in_=psf1[:], func=mybir.ActivationFunctionType.Copy
                )
                nc.gpsimd.dma_start(y2t[sl, :], y2c[:])
                nc.gpsimd.dma_start(y1t[sl, :], y1c[:])

    nc.compile()
    return nc


def make_in_maps(x2, x1, q_w, q_b, k_w, k_b, v_w, v_b, gamma):
    x2 = np.asarray(x2, dtype=np.float32)
    x1 = np.asarray(x1, dtype=np.float32)
    g = float(np.asarray(gamma).reshape(-1)[0])
    bf16 = ml_dtypes.bfloat16
    qkw = np.concatenate([np.asarray(q_w).T, np.asarray(k_w).T], axis=1).astype(
        np.float32
    )  # [C, 128]
    qkb = np.concatenate([np.asarray(q_b), np.asarray(k_b)]).reshape(1, 128).astype(
        np.float32
    )
    vwtb = np.asarray(v_w).T.astype(np.float32).astype(bf16)  # [C, C]
    gbv = (g * np.asarray(v_b)).astype(np.float32)  # [C]

    def chunked(xf):
        # [C, S] -> [NCH, KC, 128, 128]
        return np.ascontiguousarray(
            xf.reshape(KC, 128, NCH, 128).transpose(2, 0, 1, 3)
        )

    in_maps = []
    for b in range(B):
        x2fl = x2[b].reshape(C, S)
        x1fl = x1[b].reshape(C, S)
        in_maps.append(
            {
                "x2f": np.ascontiguousarray(
                    x2fl.reshape(KC, 128, NB, 512).transpose(2, 0, 1, 3)
                ),
                "x2b": chunked(x2fl.astype(bf16)),
                "x1b": chunked(x1fl.astype(bf16)),
                "xt2": np.ascontiguousarray((x2fl.T + gbv[None, :]).astype(bf16)),
                "xt1": np.ascontiguousarray((x1fl.T + gbv[None, :]).astype(bf16)),
                "qkw": qkw,
                "qkb": qkb,
                "vwtb": vwtb,
            }
        )

    return in_maps, g


def assemble_outputs(res):
    y2 = np.empty((B, C, H, W), np.float32)
    y1 = np.empty((B, C, H, W), np.float32)
    for b in range(B):
        y2[b] = unpermute(np.asarray(res[b]["y2t"]))
        y1[b] = unpermute(np.asarray(res[b]["y1t"]))
    return y2, y1


def unpermute(yt):
    # yt row = 128*w2 + 64*wp + 32*e + m for query h = 2m+e, w = 2*w2+wp
    t = yt.reshape(W // 2, 2, 2, 32, C)  # [w2, wp, e, m, c]
    return np.ascontiguousarray(
        t.transpose(4, 3, 2, 0, 1).reshape(C, H, W)
    )


def kernel(x2, x1, q_w, q_b, k_w, k_b, v_w, v_b, gamma):
    in_maps, g = make_in_maps(x2, x1, q_w, q_b, k_w, k_b, v_w, v_b, gamma)
    key = round(g, 12)
    if key not in _CACHED:
        _CACHED[key] = build_nc(g)
    nc = _CACHED[key]
    res = run_bass_kernel_spmd(nc, in_maps, list(range(B))).results
    return assemble_outputs(res)



# revision 7
# speedup vs baseline: 1.2806x; 1.2806x over previous
"""Criss-cross attention (CC module) Trainium2 Bass kernel, v3.

Shapes (full): x2,x1 [8, 512, 64, 64] fp32; q_w,k_w [64, 512]; v_w [512, 512];
biases; gamma [1]. Outputs (y2, y1) same shape as x2/x1.

Distribution: data-parallel over batch B=8, one batch element per NeuronCore.

Per-core algorithm (C=512, CQ=64, H=W=64, S=4096), all bf16 matmuls with
fp32 PSUM accumulation:
  Phase B: stream x2,x1 (channel-major, chunk-interleaved); q|k = qkw^T x2
    (+bias via activation; k stored with columns reordered (e,m,w) so the
    H-energy lhsT is a single-stride AP); vT chunks = x^T (g*v_w^T) for both
    tensors, written into the resident vp store (h-major rows s=h*64+w,
    per-chunk columns [v2|v1|p2|p1]).
  Phase C: energies as 64x64 blocks; exp() written into block-DIAGONAL att
    tiles (128x128 blocks, zero off-diagonal halves) so each att.v matmul
    runs a full K=128 contraction:
      att1[block t=w//2]: H-attention, rows (wp, e', m'), cols (wp, e, m)
      att2[block j=h//2]: W-attention, rows (hp, W'), cols (hp, w)
  Phase D: per-query normalizer sums via N=1 matmuls in w-major query order
    (att1 block colsums + 4 strided att2 column-gather colsums into PSUM
    partition quadrants), reciprocal on DVE -> rw[128,32].
  Phase E (pass 1): part = att2 . v per h-major chunk (one K=128 matmul per
    tensor), PSUM->SBUF on DVE into the vp store (unnormalized, bf16).
  Phase F (pass 2): gather w-major [v2|v1|p2|p1] chunks from the vp store
    (4 DMAs per chunk, single-source-partition fan-out, 4KB runs);
    psum = ident.p + att1.v; y = psum * rw (per-partition scale) -> bf16
    -> DRAM (partition-major).

gamma is folded into the v-weights on the host; the residual x + g*v_b is
added on the host in fp32. exp() needs no max-shift (|E|max ~ 63 << 88).
"""

import numpy as np
import ml_dtypes

import concourse.bass as bass
import concourse.mybir as mybir
import concourse.tile as tile
from concourse import bacc
from concourse.bass_utils import run_bass_kernel_spmd
from concourse.masks import make_identity

BF16 = mybir.dt.bfloat16
F32 = mybir.dt.float32

B, C, H, W = 8, 512, 64, 64
CQ = 64
S = H * W  # 4096
NCH = S // 128  # 32 spatial chunks of 128 rows
KC = C // 128  # 4 contraction chunks
NG = 8  # x stream groups (4 chunks each)

_CACHED = []


def build_nc():
    nc = bacc.Bacc("TRN2", target_bir_lowering=False, debug=False)

    xv2 = nc.dram_tensor("xv2", [NG, 128, 2048], BF16, kind="ExternalInput")
    xv1 = nc.dram_tensor("xv1", [NG, 128, 2048], BF16, kind="ExternalInput")
    wblob = nc.dram_tensor("wblob", [128, KC * 640], BF16, kind="ExternalInput")
    qkb = nc.dram_tensor("qkb", [128, 1], F32, kind="ExternalInput")

    ydram = nc.dram_tensor("ydram", [128, NCH * 1024], BF16, kind="ExternalOutput")

    Exp = mybir.ActivationFunctionType.Exp
    Ident = mybir.ActivationFunctionType.Identity

    with tile.TileContext(nc) as tc:
        with (
            tc.tile_pool(name="persist", bufs=1) as pp,
            tc.tile_pool(name="ring", bufs=2) as ring,
            tc.tile_pool(name="psum", bufs=7, space="PSUM") as psp,
        ):
            # ---- persistent tiles ----
            wt = pp.tile([128, KC * 640], BF16, tag="wt", name="wt")
            qkb_t = pp.tile([128, 1], F32, tag="qkb", name="qkb")
            q_t = pp.tile([64, S], BF16, tag="q_t", name="q_t")
            # k_r columns ordered (e, m, w): col = 2048e + 64m + w
            k_r = pp.tile([64, S], BF16, tag="k_r", name="k_r")
            att1 = pp.tile([128, S], BF16, tag="att1", name="att1")
            att2 = pp.tile([128, S], BF16, tag="att2", name="att2")
            # vp store: h-major chunk j cols [2048j : 2048(j+1)] = [v2|v1|p2|p1]
            vp = pp.tile([128, NCH * 2048], BF16, tag="vp", name="vp")
            ident = pp.tile([128, 128], BF16, tag="ident", name="ident")
            ones_col = pp.tile([128, 1], BF16, tag="ones", name="ones")
            rw = pp.tile([128, NCH], F32, tag="rw", name="rw")

            nc.gpsimd.memset(ones_col[:], 1.0)
            nc.gpsimd.memset(att1[:], 0.0)
            nc.gpsimd.memset(att2[:], 0.0)
            make_identity(nc, ident[:])

            nc.sync.dma_start(wt[:], wblob[:, :])
            nc.sync.dma_start(qkb_t[:], qkb[:, :])

            def wq(kc):
                return wt[:, 640 * kc : 640 * kc + 128]

            def wv(kc):
                return wt[:, 640 * kc + 128 : 640 * (kc + 1)]

            kr_v = k_r[:].rearrange("p (e m w) -> p m e w", e=2, w=W)

            # ---- Phase B: stream x; q/k projection + v projection ----
            for g in range(NG):
                xg2 = ring.tile([128, 2048], BF16, tag="x2", bufs=2, name="xg2")
                nc.sync.dma_start(xg2[:], xv2[g, :, :])
                xg1 = ring.tile([128, 2048], BF16, tag="x1", bufs=2, name="xg1")
                nc.sync.dma_start(xg1[:], xv1[g, :, :])

                xg2_v = xg2[:].rearrange("p (j k s) -> p k j s", k=KC, s=128)
                psqk = psp.tile([128, 512], F32, tag="ps", name="ps_qk")
                for kc in range(KC):
                    nc.tensor.matmul(
                        psqk[:],
                        lhsT=wq(kc),
                        rhs=xg2_v[:, kc, :, :],
                        start=(kc == 0),
                        stop=(kc == KC - 1),
                    )
                nc.scalar.activation(
                    out=q_t[:, 512 * g : 512 * (g + 1)], in_=psqk[0:64, :],
                    func=Ident, bias=qkb_t[0:64, 0:1],
                )
                psk_v = psqk[64:128, :].rearrange("p (j e w) -> p j e w", e=2, w=W)
                nc.scalar.activation(
                    out=kr_v[:, 4 * g : 4 * (g + 1), :, :], in_=psk_v,
                    func=Ident, bias=qkb_t[64:128, 0:1],
                )

                for j4 in range(4):
                    j = 4 * g + j4
                    for t, xg in ((0, xg2), (1, xg1)):
                        psv = psp.tile([128, 512], F32, tag="ps", name="ps_v")
                        for kc in range(KC):
                            nc.tensor.matmul(
                                psv[:],
                                lhsT=xg[
                                    :, 512 * j4 + 128 * kc : 512 * j4 + 128 * (kc + 1)
                                ],
                                rhs=wv(kc),
                                start=(kc == 0),
                                stop=(kc == KC - 1),
                            )
                        nc.vector.tensor_scalar_mul(
                            vp[:, 2048 * j + 512 * t : 2048 * j + 512 * (t + 1)],
                            psv[:],
                            1.0,
                        )

            # views: s = h*64+w = 128m + 64e + w  (h = 2m+e)
            q_em = q_t[:].rearrange("p (m e w) -> p e m w", e=2, w=W)

            # ---- Phase C: energies + exp into block-diagonal att tiles ----
            for g8 in range(8):
                psH = psp.tile([64, 512], F32, tag="ps", name="ps_eh")
                psW = psp.tile([64, 512], F32, tag="ps", name="ps_ew")
                for i in range(8):
                    w = 8 * g8 + i
                    # keys (e', m') via k_r stride-64 slice
                    nc.tensor.matmul(
                        psH[:, 64 * i : 64 * (i + 1)],
                        lhsT=k_r[:].rearrange("p (em w) -> p w em", w=W)[:, w, :],
                        rhs=q_em[:, :, :, w],
                        start=True,
                        stop=True,
                    )
                    h = w
                    off = 64 * (32 * (h % 2) + h // 2)
                    nc.tensor.matmul(
                        psW[:, 64 * i : 64 * (i + 1)],
                        lhsT=k_r[:, off : off + 64],
                        rhs=q_t[:, 64 * h : 64 * (h + 1)],
                        start=True,
                        stop=True,
                    )
                psH_v = psH[:].rearrange("p (i2 par q) -> p par i2 q", par=2, q=64)
                psW_v = psW[:].rearrange("p (i2 par q) -> p par i2 q", par=2, q=64)
                for par in range(2):
                    rows = slice(64 * par, 64 * (par + 1))
                    a1v = att1[rows, :].rearrange(
                        "p (t par2 q) -> p t par2 q", par2=2, q=64
                    )
                    a2v = att2[rows, :].rearrange(
                        "p (t par2 q) -> p t par2 q", par2=2, q=64
                    )
                    nc.scalar.activation(
                        out=a1v[:, 4 * g8 : 4 * (g8 + 1), par, :],
                        in_=psH_v[:, par, :, :],
                        func=Exp,
                    )
                    nc.scalar.activation(
                        out=a2v[:, 4 * g8 : 4 * (g8 + 1), par, :],
                        in_=psW_v[:, par, :, :],
                        func=Exp,
                    )

            # ---- Phase D: normalizer sums (w-major query order) ----
            # att2 col = h*64+w = 128m + 64e + 2jw + wp; for fixed (wp,e)
            # the 32 m-columns form a single-stride AP (stride 128).
            att2v = att2[:].rearrange("p (m e jw wp) -> p jw wp e m", m=32, e=2, wp=2)
            psR = psp.tile([128, NCH], F32, tag="psR", bufs=1, name="ps_r")
            for jp in range(NCH):
                nc.tensor.matmul(
                    psR[:, jp : jp + 1],
                    lhsT=att1[:, 128 * jp : 128 * (jp + 1)],
                    rhs=ones_col[:],
                    start=True,
                    stop=False,
                    skip_group_check=True,
                )
                for wp in range(2):
                    for e in range(2):
                        rows = slice(64 * wp + 32 * e, 64 * wp + 32 * (e + 1))
                        nc.tensor.matmul(
                            psR[rows, jp : jp + 1],
                            lhsT=att2v[:, jp, wp, e, :],
                            rhs=ones_col[:],
                            start=False,
                            stop=(wp == 1 and e == 1),
                            skip_group_check=True,
                            tile_position=(0, 64 * wp + 32 * e),
                        )
            nc.vector.reciprocal(rw[:], psR[:])

            # ---- Phase E: pass 1 (W-attention, h-major chunks) ----
            for j in range(NCH):
                for t in range(2):
                    pso = psp.tile([128, 512], F32, tag="ps", name="ps_o")
                    nc.tensor.matmul(
                        pso[:],
                        lhsT=att2[:, 128 * j : 128 * (j + 1)],
                        rhs=vp[:, 2048 * j + 512 * t : 2048 * j + 512 * (t + 1)],
                        start=True,
                        stop=True,
                    )
                    nc.vector.tensor_scalar_mul(
                        vp[
                            :,
                            2048 * j + 1024 + 512 * t : 2048 * j + 1024 + 512 * (t + 1),
                        ],
                        pso[:],
                        1.0,
                    )

            # ---- Phase F: pass 2 (H-attention, w-major chunks) ----
            # vp src: partition (e, w), free (m, qc=2048); gather per (e, wp)
            # into contiguous 32-row blocks (rows (wp, e, m)), 4KB runs.
            vp_v = vp[:].rearrange("(e w) (m qc) -> e w m qc", e=2, qc=2048)
            for jp in range(NCH):
                vpc = ring.tile([128, 2048], BF16, tag="vpc", bufs=3, name="vpc")
                for e in range(2):
                    for wp in range(2):
                        rows = slice(64 * wp + 32 * e, 64 * wp + 32 * (e + 1))
                        eng = nc.sync if e == 0 else nc.scalar
                        eng.dma_start(
                            vpc[rows, :],
                            vp_v[e, 2 * jp + wp : 2 * jp + wp + 1, :, :],
                        )

                yt = ring.tile([128, 1024], BF16, tag="y", bufs=3, name="yt")
                for t in range(2):
                    psf = psp.tile([128, 512], F32, tag="ps", name="ps_f")
                    nc.tensor.matmul(
                        psf[:],
                        lhsT=ident[:],
                        rhs=vpc[:, 1024 + 512 * t : 1024 + 512 * (t + 1)],
                        start=True,
                        stop=False,
                        skip_group_check=True,
                    )
                    nc.tensor.matmul(
                        psf[:],
                        lhsT=att1[:, 128 * jp : 128 * (jp + 1)],
                        rhs=vpc[:, 512 * t : 512 * (t + 1)],
                        start=False,
                        stop=True,
                        skip_group_check=True,
                    )
                    nc.scalar.activation(
                        out=yt[:, 512 * t : 512 * (t + 1)], in_=psf[:], func=Ident,
                        scale=rw[:, jp : jp + 1],
                    )
                nc.sync.dma_start(ydram[:, 1024 * jp : 1024 * (jp + 1)], yt[:])

    nc.compile()
    return nc


def make_in_maps(x2, x1, q_w, q_b, k_w, k_b, v_w, v_b, gamma):
    x2 = np.asarray(x2, dtype=np.float32)
    x1 = np.asarray(x1, dtype=np.float32)
    g = float(np.asarray(gamma).reshape(-1)[0])
    bf16 = ml_dtypes.bfloat16

    # wblob: per kc, [qkw chunk | g * v_w^T chunk]
    qkw = np.concatenate([np.asarray(q_w).T, np.asarray(k_w).T], axis=1)  # [C,128]
    gvwt = (g * np.asarray(v_w).T).astype(np.float32)  # [C, C]
    wb = np.empty((128, KC * 640), np.float32)
    for kc in range(KC):
        wb[:, 640 * kc : 640 * kc + 128] = qkw[128 * kc : 128 * (kc + 1), :]
        wb[:, 640 * kc + 128 : 640 * (kc + 1)] = gvwt[128 * kc : 128 * (kc + 1), :]
    wb = wb.astype(bf16)
    qkb = (
        np.concatenate([np.asarray(q_b), np.asarray(k_b)])
        .reshape(128, 1)
        .astype(np.float32)
    )

    def pack_x(xfl):
        # [C, S] -> [NG, 128, 2048]: [g, p, j4*512 + kc*128 + sl]
        t = xfl.reshape(KC, 128, NG, 4, 128)  # [kc, p, g, j4, sl]
        return np.ascontiguousarray(
            t.transpose(2, 1, 3, 0, 4).reshape(NG, 128, 2048).astype(bf16)
        )

    in_maps = []
    for b in range(B):
        in_maps.append(
            {
                "xv2": pack_x(x2[b].reshape(C, S)),
                "xv1": pack_x(x1[b].reshape(C, S)),
                "wblob": wb,
                "qkb": qkb,
            }
        )

    # host-side residual (fp32): x + g*v_b
    gvb = (g * np.asarray(v_b)).astype(np.float32)[None, :, None, None]
    return in_maps, (x2 + gvb, x1 + gvb)


def assemble_outputs(res, resid):
    r2, r1 = resid
    y2 = np.empty((B, C, H, W), np.float32)
    y1 = np.empty((B, C, H, W), np.float32)
    for b in range(B):
        yd = np.asarray(res[b]["ydram"])  # [128, NCH*1024] bf16
        # row p = wp*64 + e*32 + m ; col = jp*1024 + t*512 + c
        t = yd.reshape(2, 2, 32, NCH, 2, 512).astype(np.float32)
        # [wp, e, m, jp, t, c] -> y[t][c, h=2m+e, w=2jp+wp]
        t = t.transpose(4, 5, 2, 1, 3, 0)  # [t, c, m, e, jp, wp]
        y2[b] = t[0].reshape(C, H, W)
        y1[b] = t[1].reshape(C, H, W)
    y2 += r2
    y1 += r1
    return y2, y1


def kernel(x2, x1, q_w, q_b, k_w, k_b, v_w, v_b, gamma):
    in_maps, resid = make_in_maps(x2, x1, q_w, q_b, k_w, k_b, v_w, v_b, gamma)
    if not _CACHED:
        _CACHED.append(build_nc())
    nc = _CACHED[0]
    res = run_bass_kernel_spmd(nc, in_maps, list(range(B))).results
    return assemble_outputs(res, resid)


# revision 12
# speedup vs baseline: 1.4344x; 1.1201x over previous
"""Criss-cross attention (CC module) Trainium2 Bass kernel, v3.

Shapes (full): x2,x1 [8, 512, 64, 64] fp32; q_w,k_w [64, 512]; v_w [512, 512];
biases; gamma [1]. Outputs (y2, y1) same shape as x2/x1.

Distribution: data-parallel over batch B=8, one batch element per NeuronCore.

Per-core algorithm (C=512, CQ=64, H=W=64, S=4096), all bf16 matmuls with
fp32 PSUM accumulation:
  Phase B: stream x2,x1 (channel-major, chunk-interleaved); q|k = qkw^T x2
    (+bias via activation; k stored with columns reordered (e,m,w) so the
    H-energy lhsT is a single-stride AP); vT chunks = x^T (g*v_w^T) for both
    tensors, written into the resident vp store (h-major rows s=h*64+w,
    per-chunk columns [v2|v1|p2|p1]).
  Phase C: energies as 64x64 blocks; exp() written into block-DIAGONAL att
    tiles (128x128 blocks, zero off-diagonal halves) so each att.v matmul
    runs a full K=128 contraction:
      att1[block t=w//2]: H-attention, rows (wp, e', m'), cols (wp, e, m)
      att2[block j=h//2]: W-attention, rows (hp, W'), cols (hp, w)
  Phase D: per-query normalizer sums via N=1 matmuls in w-major query order
    (att1 block colsums + 4 strided att2 column-gather colsums into PSUM
    partition quadrants), reciprocal on DVE -> rw[128,32].
  Phase E (pass 1): part = att2 . v per h-major chunk (one K=128 matmul per
    tensor), PSUM->SBUF on DVE into the vp store (unnormalized, bf16).
  Phase F (pass 2): gather w-major [v2|v1|p2|p1] chunks from the vp store
    (4 DMAs per chunk, single-source-partition fan-out, 4KB runs);
    psum = ident.p + att1.v; y = psum * rw (per-partition scale) -> bf16
    -> DRAM (partition-major).

gamma is folded into the v-weights on the host; the residual x + g*v_b is
added on the host in fp32. exp() needs no max-shift (|E|max ~ 63 << 88).
"""

import numpy as np
import ml_dtypes

import concourse.bass as bass
import concourse.mybir as mybir
import concourse.tile as tile
from concourse import bacc
from concourse.bass_utils import run_bass_kernel_spmd
from concourse.masks import make_identity

BF16 = mybir.dt.bfloat16
F32 = mybir.dt.float32

B, C, H, W = 8, 512, 64, 64
CQ = 64
S = H * W  # 4096
NCH = S // 128  # 32 spatial chunks of 128 rows
KC = C // 128  # 4 contraction chunks
NG = 8  # x stream groups (4 chunks each)

_CACHED = []


def build_nc():
    nc = bacc.Bacc("TRN2", target_bir_lowering=False, debug=False)

    xv2 = nc.dram_tensor("xv2", [NG, 128, 2048], BF16, kind="ExternalInput")
    xv1 = nc.dram_tensor("xv1", [NG, 128, 2048], BF16, kind="ExternalInput")
    wblob = nc.dram_tensor("wblob", [128, KC * 640], BF16, kind="ExternalInput")
    qkb = nc.dram_tensor("qkb", [128, 1], F32, kind="ExternalInput")

    ydram = nc.dram_tensor("ydram", [128, NCH * 1024], BF16, kind="ExternalOutput")

    # internal DRAM staging for the h-major -> w-major crossing (pass 1 ->
    # pass 2): chunk-major [v2|v1|p2|p1] rows, read back with a strided AP.
    vp_dram = nc.dram_tensor("vp_dram", [NCH, 128, 2048], BF16)

    Exp = mybir.ActivationFunctionType.Exp
    Ident = mybir.ActivationFunctionType.Identity

    with tile.TileContext(nc) as tc:
        with (
            tc.tile_pool(name="persist", bufs=1) as pp,
            tc.tile_pool(name="ring", bufs=2) as ring,
            tc.tile_pool(name="psum", bufs=7, space="PSUM") as psp,
        ):
            # ---- persistent tiles ----
            wt = pp.tile([128, KC * 640], BF16, tag="wt", name="wt")
            qkb_t = pp.tile([128, 1], F32, tag="qkb", name="qkb")
            q_t = pp.tile([64, S], BF16, tag="q_t", name="q_t")
            # k_r columns ordered (e, m, w): col = 2048e + 64m + w
            k_r = pp.tile([64, S], BF16, tag="k_r", name="k_r")
            att1 = pp.tile([128, S], BF16, tag="att1", name="att1")
            att2 = pp.tile([128, S], BF16, tag="att2", name="att2")
            # vp store: h-major chunk j cols [2048j : 2048(j+1)] = [v2|v1|p2|p1]
            vp = pp.tile([128, NCH * 2048], BF16, tag="vp", name="vp")
            ident = pp.tile([128, 128], BF16, tag="ident", name="ident")
            ones_col = pp.tile([128, 1], BF16, tag="ones", name="ones")
            rw = pp.tile([128, NCH], F32, tag="rw", name="rw")

            nc.gpsimd.memset(ones_col[:], 1.0)
            nc.gpsimd.memset(att1[:], 0.0)
            nc.gpsimd.memset(att2[:], 0.0)
            make_identity(nc, ident[:])

            nc.sync.dma_start(wt[:], wblob[:, :])
            nc.sync.dma_start(qkb_t[:], qkb[:, :])

            def wq(kc):
                return wt[:, 640 * kc : 640 * kc + 128]

            def wv(kc):
                return wt[:, 640 * kc + 128 : 640 * (kc + 1)]

            kr_v = k_r[:].rearrange("p (e m w) -> p m e w", e=2, w=W)

            # ---- Phase B: stream x; q/k projection + v projection ----
            for g in range(NG):
                xg2 = ring.tile([128, 2048], BF16, tag="x2", bufs=2, name="xg2")
                nc.sync.dma_start(xg2[:], xv2[g, :, :])
                xg1 = ring.tile([128, 2048], BF16, tag="x1", bufs=2, name="xg1")
                nc.sync.dma_start(xg1[:], xv1[g, :, :])

                xg2_v = xg2[:].rearrange("p (j k s) -> p k j s", k=KC, s=128)
                psqk = psp.tile([128, 512], F32, tag="ps", name="ps_qk")
                for kc in range(KC):
                    nc.tensor.matmul(
                        psqk[:],
                        lhsT=wq(kc),
                        rhs=xg2_v[:, kc, :, :],
                        start=(kc == 0),
                        stop=(kc == KC - 1),
                    )
                nc.scalar.activation(
                    out=q_t[:, 512 * g : 512 * (g + 1)], in_=psqk[0:64, :],
                    func=Ident, bias=qkb_t[0:64, 0:1],
                )
                psk_v = psqk[64:128, :].rearrange("p (j e w) -> p j e w", e=2, w=W)
                nc.scalar.activation(
                    out=kr_v[:, 4 * g : 4 * (g + 1), :, :], in_=psk_v,
                    func=Ident, bias=qkb_t[64:128, 0:1],
                )

                for j4 in range(4):
                    j = 4 * g + j4
                    for t, xg in ((0, xg2), (1, xg1)):
                        psv = psp.tile([128, 512], F32, tag="ps", name="ps_v")
                        for kc in range(KC):
                            nc.tensor.matmul(
                                psv[:],
                                lhsT=xg[
                                    :, 512 * j4 + 128 * kc : 512 * j4 + 128 * (kc + 1)
                                ],
                                rhs=wv(kc),
                                start=(kc == 0),
                                stop=(kc == KC - 1),
                            )
                        nc.vector.tensor_scalar_mul(
                            vp[:, 2048 * j + 512 * t : 2048 * j + 512 * (t + 1)],
                            psv[:],
                            1.0,
                        )

            # views: s = h*64+w = 128m + 64e + w  (h = 2m+e)
            q_em = q_t[:].rearrange("p (m e w) -> p e m w", e=2, w=W)

            # ---- Phase C: energies + exp into block-diagonal att tiles ----
            for g8 in range(8):
                psH = psp.tile([64, 512], F32, tag="ps", name="ps_eh")
                psW = psp.tile([64, 512], F32, tag="ps", name="ps_ew")
                for i in range(8):
                    w = 8 * g8 + i
                    # keys (e', m') via k_r stride-64 slice
                    nc.tensor.matmul(
                        psH[:, 64 * i : 64 * (i + 1)],
                        lhsT=k_r[:].rearrange("p (em w) -> p w em", w=W)[:, w, :],
                        rhs=q_em[:, :, :, w],
                        start=True,
                        stop=True,
                    )
                    h = w
                    off = 64 * (32 * (h % 2) + h // 2)
                    nc.tensor.matmul(
                        psW[:, 64 * i : 64 * (i + 1)],
                        lhsT=k_r[:, off : off + 64],
                        rhs=q_t[:, 64 * h : 64 * (h + 1)],
                        start=True,
                        stop=True,
                    )
                psH_v = psH[:].rearrange("p (i2 par q) -> p par i2 q", par=2, q=64)
                psW_v = psW[:].rearrange("p (i2 par q) -> p par i2 q", par=2, q=64)
                for par in range(2):
                    rows = slice(64 * par, 64 * (par + 1))
                    a1v = att1[rows, :].rearrange(
                        "p (t par2 q) -> p t par2 q", par2=2, q=64
                    )
                    a2v = att2[rows, :].rearrange(
                        "p (t par2 q) -> p t par2 q", par2=2, q=64
                    )
                    nc.scalar.activation(
                        out=a1v[:, 4 * g8 : 4 * (g8 + 1), par, :],
                        in_=psH_v[:, par, :, :],
                        func=Exp,
                    )
                    nc.scalar.activation(
                        out=a2v[:, 4 * g8 : 4 * (g8 + 1), par, :],
                        in_=psW_v[:, par, :, :],
                        func=Exp,
                    )

            # ---- Phase D: normalizer sums (w-major query order) ----
            # att2 col = h*64+w = 128m + 64e + 2jw + wp; for fixed (wp,e)
            # the 32 m-columns form a single-stride AP (stride 128).
            att2v = att2[:].rearrange("p (m e jw wp) -> p jw wp e m", m=32, e=2, wp=2)
            psR = psp.tile([128, NCH], F32, tag="psR", bufs=1, name="ps_r")
            for jp in range(NCH):
                nc.tensor.matmul(
                    psR[:, jp : jp + 1],
                    lhsT=att1[:, 128 * jp : 128 * (jp + 1)],
                    rhs=ones_col[:],
                    start=True,
                    stop=False,
                    skip_group_check=True,
                )
                for wp in range(2):
                    for e in range(2):
                        rows = slice(64 * wp + 32 * e, 64 * wp + 32 * (e + 1))
                        nc.tensor.matmul(
                            psR[rows, jp : jp + 1],
                            lhsT=att2v[:, jp, wp, e, :],
                            rhs=ones_col[:],
                            start=False,
                            stop=(wp == 1 and e == 1),
                            skip_group_check=True,
                            tile_position=(0, 64 * wp + 32 * e),
                        )
            nc.vector.reciprocal(rw[:], psR[:])

            # ---- Phase E: pass 1 (W-attention, h-major chunks) ----
            for j in range(NCH):
                for t in range(2):
                    pso = psp.tile([128, 512], F32, tag="ps", name="ps_o")
                    nc.tensor.matmul(
                        pso[:],
                        lhsT=att2[:, 128 * j : 128 * (j + 1)],
                        rhs=vp[:, 2048 * j + 512 * t : 2048 * j + 512 * (t + 1)],
                        start=True,
                        stop=True,
                    )
                    nc.vector.tensor_scalar_mul(
                        vp[
                            :,
                            2048 * j + 1024 + 512 * t : 2048 * j + 1024 + 512 * (t + 1),
                        ],
                        pso[:],
                        1.0,
                    )
                nc.sync.dma_start(
                    vp_dram[j, :, :], vp[:, 2048 * j : 2048 * (j + 1)]
                )

            # ---- Phase F: pass 2 (H-attention, w-major chunks) ----
            # read w-major chunks back from DRAM: dst row (wp, e, m) <-
            # vp_dram[m, 64e + 2jp + wp, :], 4KB runs, partition-parallel.
            vpd_v = vp_dram.rearrange("m (e w) qc -> w e m qc", e=2)
            for jp in range(NCH):
                vpc = ring.tile([128, 2048], BF16, tag="vpc", bufs=3, name="vpc")
                for wp in range(2):
                    nc.sync.dma_start(
                        vpc[64 * wp : 64 * (wp + 1), :],
                        vpd_v[2 * jp + wp, :, :, :],
                    )

                yt = ring.tile([128, 1024], BF16, tag="y", bufs=3, name="yt")
                for t in range(2):
                    psf = psp.tile([128, 512], F32, tag="ps", name="ps_f")
                    nc.tensor.matmul(
                        psf[:],
                        lhsT=ident[:],
                        rhs=vpc[:, 1024 + 512 * t : 1024 + 512 * (t + 1)],
                        start=True,
                        stop=False,
                        skip_group_check=True,
                    )
                    nc.tensor.matmul(
                        psf[:],
                        lhsT=att1[:, 128 * jp : 128 * (jp + 1)],
                        rhs=vpc[:, 512 * t : 512 * (t + 1)],
                        start=False,
                        stop=True,
                        skip_group_check=True,
                    )
                    nc.scalar.activation(
                        out=yt[:, 512 * t : 512 * (t + 1)], in_=psf[:], func=Ident,
                        scale=rw[:, jp : jp + 1],
                    )
                nc.scalar.dma_start(ydram[:, 1024 * jp : 1024 * (jp + 1)], yt[:])

    nc.compile()
    return nc


def make_in_maps(x2, x1, q_w, q_b, k_w, k_b, v_w, v_b, gamma):
    x2 = np.asarray(x2, dtype=np.float32)
    x1 = np.asarray(x1, dtype=np.float32)
    g = float(np.asarray(gamma).reshape(-1)[0])
    bf16 = ml_dtypes.bfloat16

    # wblob: per kc, [qkw chunk | g * v_w^T chunk]
    qkw = np.concatenate([np.asarray(q_w).T, np.asarray(k_w).T], axis=1)  # [C,128]
    gvwt = (g * np.asarray(v_w).T).astype(np.float32)  # [C, C]
    wb = np.empty((128, KC * 640), np.float32)
    for kc in range(KC):
        wb[:, 640 * kc : 640 * kc + 128] = qkw[128 * kc : 128 * (kc + 1), :]
        wb[:, 640 * kc + 128 : 640 * (kc + 1)] = gvwt[128 * kc : 128 * (kc + 1), :]
    wb = wb.astype(bf16)
    qkb = (
        np.concatenate([np.asarray(q_b), np.asarray(k_b)])
        .reshape(128, 1)
        .astype(np.float32)
    )

    def pack_x(xfl):
        # [C, S] -> [NG, 128, 2048]: [g, p, j4*512 + kc*128 + sl]
        t = xfl.reshape(KC, 128, NG, 4, 128)  # [kc, p, g, j4, sl]
        return np.ascontiguousarray(
            t.transpose(2, 1, 3, 0, 4).reshape(NG, 128, 2048).astype(bf16)
        )

    in_maps = []
    for b in range(B):
        in_maps.append(
            {
                "xv2": pack_x(x2[b].reshape(C, S)),
                "xv1": pack_x(x1[b].reshape(C, S)),
                "wblob": wb,
                "qkb": qkb,
            }
        )

    # host-side residual (fp32): x + g*v_b
    gvb = (g * np.asarray(v_b)).astype(np.float32)[None, :, None, None]
    return in_maps, (x2 + gvb, x1 + gvb)


def assemble_outputs(res, resid):
    r2, r1 = resid
    y2 = np.empty((B, C, H, W), np.float32)
    y1 = np.empty((B, C, H, W), np.float32)
    for b in range(B):
        yd = np.asarray(res[b]["ydram"])  # [128, NCH*1024] bf16
        # row p = wp*64 + e*32 + m ; col = jp*1024 + t*512 + c
        t = yd.reshape(2, 2, 32, NCH, 2, 512).astype(np.float32)
        # [wp, e, m, jp, t, c] -> y[t][c, h=2m+e, w=2jp+wp]
        t = t.transpose(4, 5, 2, 1, 3, 0)  # [t, c, m, e, jp, wp]
        y2[b] = t[0].reshape(C, H, W)
        y1[b] = t[1].reshape(C, H, W)
    y2 += r2
    y1 += r1
    return y2, y1


def kernel(x2, x1, q_w, q_b, k_w, k_b, v_w, v_b, gamma):
    in_maps, resid = make_in_maps(x2, x1, q_w, q_b, k_w, k_b, v_w, v_b, gamma)
    if not _CACHED:
        _CACHED.append(build_nc())
    nc = _CACHED[0]
    res = run_bass_kernel_spmd(nc, in_maps, list(range(B))).results
    return assemble_outputs(res, resid)


# revision 13
# speedup vs baseline: 2.2949x; 1.5999x over previous
"""Criss-cross attention (CC module) Trainium2 Bass kernel, v3.

Shapes (full): x2,x1 [8, 512, 64, 64] fp32; q_w,k_w [64, 512]; v_w [512, 512];
biases; gamma [1]. Outputs (y2, y1) same shape as x2/x1.

Distribution: data-parallel over batch B=8, one batch element per NeuronCore.

Per-core algorithm (C=512, CQ=64, H=W=64, S=4096), all bf16 matmuls with
fp32 PSUM accumulation:
  Phase B: stream x2,x1 (channel-major, chunk-interleaved); q|k = qkw^T x2
    (+bias via activation; k stored with columns reordered (e,m,w) so the
    H-energy lhsT is a single-stride AP); vT chunks = x^T (g*v_w^T) for both
    tensors, written into the resident vp store (h-major rows s=h*64+w,
    per-chunk columns [v2|v1|p2|p1]).
  Phase C: energies as 64x64 blocks; exp() written into block-DIAGONAL att
    tiles (128x128 blocks, zero off-diagonal halves) so each att.v matmul
    runs a full K=128 contraction:
      att1[block t=w//2]: H-attention, rows (wp, e', m'), cols (wp, e, m)
      att2[block j=h//2]: W-attention, rows (hp, W'), cols (hp, w)
  Phase D: per-query normalizer sums via N=1 matmuls in w-major query order
    (att1 block colsums + 4 strided att2 column-gather colsums into PSUM
    partition quadrants), reciprocal on DVE -> rw[128,32].
  Phase E (pass 1): part = att2 . v per h-major chunk (one K=128 matmul per
    tensor), PSUM->SBUF on DVE into the vp store (unnormalized, bf16).
  Phase F (pass 2): gather w-major [v2|v1|p2|p1] chunks from the vp store
    (4 DMAs per chunk, single-source-partition fan-out, 4KB runs);
    psum = ident.p + att1.v; y = psum * rw (per-partition scale) -> bf16
    -> DRAM (partition-major).

gamma is folded into the v-weights on the host; the residual x + g*v_b is
added on the host in fp32. exp() needs no max-shift (|E|max ~ 63 << 88).
"""

import numpy as np
import ml_dtypes

import concourse.bass as bass
import concourse.mybir as mybir
import concourse.tile as tile
from concourse import bacc
from concourse.bass_utils import run_bass_kernel_spmd
from concourse.masks import make_identity

BF16 = mybir.dt.bfloat16
F32 = mybir.dt.float32

B, C, H, W = 8, 512, 64, 64
CQ = 64
S = H * W  # 4096
NCH = S // 128  # 32 spatial chunks of 128 rows
KC = C // 128  # 4 contraction chunks
NG = 8  # x stream groups (4 chunks each)

_CACHED = []


def build_nc():
    nc = bacc.Bacc("TRN2", target_bir_lowering=False, debug=False)

    xv2 = nc.dram_tensor("xv2", [NG, 128, 2048], BF16, kind="ExternalInput")
    xv1 = nc.dram_tensor("xv1", [NG, 128, 2048], BF16, kind="ExternalInput")
    wblob = nc.dram_tensor("wblob", [128, KC * 640], BF16, kind="ExternalInput")
    qkb = nc.dram_tensor("qkb", [128, 1], F32, kind="ExternalInput")

    ydram = nc.dram_tensor("ydram", [128, NCH * 1024], BF16, kind="ExternalOutput")

    # internal DRAM staging for the h-major -> w-major crossing (pass 1 ->
    # pass 2): chunk-major [v2|v1|p2|p1] rows, read back with a strided AP.
    vp_dram = nc.dram_tensor("vp_dram", [NCH, 128, 2048], BF16)

    Exp = mybir.ActivationFunctionType.Exp
    Ident = mybir.ActivationFunctionType.Identity

    with tile.TileContext(nc) as tc:
        with (
            tc.tile_pool(name="persist", bufs=1) as pp,
            tc.tile_pool(name="ring", bufs=2) as ring,
            tc.tile_pool(name="psum", bufs=7, space="PSUM") as psp,
        ):
            # ---- persistent tiles ----
            wt = pp.tile([128, KC * 640], BF16, tag="wt", name="wt")
            qkb_t = pp.tile([128, 1], F32, tag="qkb", name="qkb")
            q_t = pp.tile([64, S], BF16, tag="q_t", name="q_t")
            # k_r columns ordered (e, m, w): col = 2048e + 64m + w
            k_r = pp.tile([64, S], BF16, tag="k_r", name="k_r")
            att1 = pp.tile([128, S], BF16, tag="att1", name="att1")
            att2 = pp.tile([128, S], BF16, tag="att2", name="att2")
            # vp store: h-major chunk j cols [2048j : 2048(j+1)] = [v2|v1|p2|p1]
            vp = pp.tile([128, NCH * 2048], BF16, tag="vp", name="vp")
            ident = pp.tile([128, 128], BF16, tag="ident", name="ident")
            ones_col = pp.tile([128, 1], BF16, tag="ones", name="ones")
            rw = pp.tile([128, NCH], F32, tag="rw", name="rw")

            nc.gpsimd.memset(ones_col[:], 1.0)
            nc.gpsimd.memset(att1[:], 0.0)
            nc.gpsimd.memset(att2[:], 0.0)
            make_identity(nc, ident[:])

            nc.sync.dma_start(wt[:], wblob[:, :])
            nc.sync.dma_start(qkb_t[:], qkb[:, :])

            def wq(kc):
                return wt[:, 640 * kc : 640 * kc + 128]

            def wv(kc):
                return wt[:, 640 * kc + 128 : 640 * (kc + 1)]

            kr_v = k_r[:].rearrange("p (e m w) -> p m e w", e=2, w=W)

            # ---- Phase B: stream x; q/k projection + v projection ----
            for g in range(NG):
                xg2 = ring.tile([128, 2048], BF16, tag="x2", bufs=2, name="xg2")
                nc.sync.dma_start(xg2[:], xv2[g, :, :])
                xg1 = ring.tile([128, 2048], BF16, tag="x1", bufs=2, name="xg1")
                nc.sync.dma_start(xg1[:], xv1[g, :, :])

                xg2_v = xg2[:].rearrange("p (j k s) -> p k j s", k=KC, s=128)
                psqk = psp.tile([128, 512], F32, tag="ps", name="ps_qk")
                for kc in range(KC):
                    nc.tensor.matmul(
                        psqk[:],
                        lhsT=wq(kc),
                        rhs=xg2_v[:, kc, :, :],
                        start=(kc == 0),
                        stop=(kc == KC - 1),
                    )
                nc.scalar.activation(
                    out=q_t[:, 512 * g : 512 * (g + 1)], in_=psqk[0:64, :],
                    func=Ident, bias=qkb_t[0:64, 0:1],
                )
                psk_v = psqk[64:128, :].rearrange("p (j e w) -> p j e w", e=2, w=W)
                nc.scalar.activation(
                    out=kr_v[:, 4 * g : 4 * (g + 1), :, :], in_=psk_v,
                    func=Ident, bias=qkb_t[64:128, 0:1],
                )

                for j4 in range(4):
                    j = 4 * g + j4
                    for t, xg in ((0, xg2), (1, xg1)):
                        psv = psp.tile([128, 512], F32, tag="ps", name="ps_v")
                        for kc in range(KC):
                            nc.tensor.matmul(
                                psv[:],
                                lhsT=xg[
                                    :, 512 * j4 + 128 * kc : 512 * j4 + 128 * (kc + 1)
                                ],
                                rhs=wv(kc),
                                start=(kc == 0),
                                stop=(kc == KC - 1),
                            )
                        nc.vector.tensor_scalar_mul(
                            vp[:, 2048 * j + 512 * t : 2048 * j + 512 * (t + 1)],
                            psv[:],
                            1.0,
                        )

            # views: s = h*64+w = 128m + 64e + w  (h = 2m+e)
            q_em = q_t[:].rearrange("p (m e w) -> p e m w", e=2, w=W)

            # ---- Phase C: energies + exp into block-diagonal att tiles ----
            for g8 in range(8):
                psH = psp.tile([64, 512], F32, tag="ps", name="ps_eh")
                psW = psp.tile([64, 512], F32, tag="ps", name="ps_ew")
                for i in range(8):
                    w = 8 * g8 + i
                    # keys (e', m') via k_r stride-64 slice
                    nc.tensor.matmul(
                        psH[:, 64 * i : 64 * (i + 1)],
                        lhsT=k_r[:].rearrange("p (em w) -> p w em", w=W)[:, w, :],
                        rhs=q_em[:, :, :, w],
                        start=True,
                        stop=True,
                    )
                    h = w
                    off = 64 * (32 * (h % 2) + h // 2)
                    nc.tensor.matmul(
                        psW[:, 64 * i : 64 * (i + 1)],
                        lhsT=k_r[:, off : off + 64],
                        rhs=q_t[:, 64 * h : 64 * (h + 1)],
                        start=True,
                        stop=True,
                    )
                psH_v = psH[:].rearrange("p (i2 par q) -> p par i2 q", par=2, q=64)
                psW_v = psW[:].rearrange("p (i2 par q) -> p par i2 q", par=2, q=64)
                for par in range(2):
                    rows = slice(64 * par, 64 * (par + 1))
                    a1v = att1[rows, :].rearrange(
                        "p (t par2 q) -> p t par2 q", par2=2, q=64
                    )
                    a2v = att2[rows, :].rearrange(
                        "p (t par2 q) -> p t par2 q", par2=2, q=64
                    )
                    nc.scalar.activation(
                        out=a1v[:, 4 * g8 : 4 * (g8 + 1), par, :],
                        in_=psH_v[:, par, :, :],
                        func=Exp,
                    )
                    nc.scalar.activation(
                        out=a2v[:, 4 * g8 : 4 * (g8 + 1), par, :],
                        in_=psW_v[:, par, :, :],
                        func=Exp,
                    )

            # ---- Phase D: normalizer sums (w-major query order) ----
            # att2 col = h*64+w = 128m + 64e + 2jw + wp; for fixed (wp,e)
            # the 32 m-columns form a single-stride AP (stride 128).
            att2v = att2[:].rearrange("p (m e jw wp) -> p jw wp e m", m=32, e=2, wp=2)
            psR = psp.tile([128, NCH], F32, tag="psR", bufs=1, name="ps_r")
            for jp in range(NCH):
                nc.tensor.matmul(
                    psR[:, jp : jp + 1],
                    lhsT=att1[:, 128 * jp : 128 * (jp + 1)],
                    rhs=ones_col[:],
                    start=True,
                    stop=False,
                    skip_group_check=True,
                )
                for wp in range(2):
                    for e in range(2):
                        rows = slice(64 * wp + 32 * e, 64 * wp + 32 * (e + 1))
                        nc.tensor.matmul(
                            psR[rows, jp : jp + 1],
                            lhsT=att2v[:, jp, wp, e, :],
                            rhs=ones_col[:],
                            start=False,
                            stop=(wp == 1 and e == 1),
                            skip_group_check=True,
                            tile_position=(0, 64 * wp + 32 * e),
                        )
            nc.vector.reciprocal(rw[:], psR[:])

            # ---- Phase E: pass 1 (W-attention, h-major chunks) ----
            for j in range(NCH):
                for t in range(2):
                    pso = psp.tile([128, 512], F32, tag="ps", name="ps_o")
                    nc.tensor.matmul(
                        pso[:],
                        lhsT=att2[:, 128 * j : 128 * (j + 1)],
                        rhs=vp[:, 2048 * j + 512 * t : 2048 * j + 512 * (t + 1)],
                        start=True,
                        stop=True,
                    )
                    nc.vector.tensor_scalar_mul(
                        vp[
                            :,
                            2048 * j + 1024 + 512 * t : 2048 * j + 1024 + 512 * (t + 1),
                        ],
                        pso[:],
                        1.0,
                    )
                nc.sync.dma_start(
                    vp_dram[j, :, :], vp[:, 2048 * j : 2048 * (j + 1)]
                )

            # ---- Phase F: pass 2 (H-attention, w-major chunks) ----
            # read w-major chunks back from DRAM: dst row (wp, e, m) <-
            # vp_dram[m, 64e + 2jp + wp, :]. Each (e, wp) piece is a pure-2D
            # uniform-stride transfer so the DGE spreads it across all DMA
            # engines (multi-level APs pin to 2 engines).
            for jp in range(NCH):
                vpc = ring.tile([128, 2048], BF16, tag="vpc", bufs=3, name="vpc")
                for wp in range(2):
                    eng = nc.sync if wp == 0 else nc.scalar
                    for e in range(2):
                        rows = slice(64 * wp + 32 * e, 64 * wp + 32 * (e + 1))
                        eng.dma_start(
                            vpc[rows, :],
                            vp_dram[:, 64 * e + 2 * jp + wp, :],
                        )

                yt = ring.tile([128, 1024], BF16, tag="y", bufs=3, name="yt")
                for t in range(2):
                    psf = psp.tile([128, 512], F32, tag="ps", name="ps_f")
                    nc.tensor.matmul(
                        psf[:],
                        lhsT=ident[:],
                        rhs=vpc[:, 1024 + 512 * t : 1024 + 512 * (t + 1)],
                        start=True,
                        stop=False,
                        skip_group_check=True,
                    )
                    nc.tensor.matmul(
                        psf[:],
                        lhsT=att1[:, 128 * jp : 128 * (jp + 1)],
                        rhs=vpc[:, 512 * t : 512 * (t + 1)],
                        start=False,
                        stop=True,
                        skip_group_check=True,
                    )
                    nc.scalar.activation(
                        out=yt[:, 512 * t : 512 * (t + 1)], in_=psf[:], func=Ident,
                        scale=rw[:, jp : jp + 1],
                    )
                nc.scalar.dma_start(ydram[:, 1024 * jp : 1024 * (jp + 1)], yt[:])

    nc.compile()
    return nc


def make_in_maps(x2, x1, q_w, q_b, k_w, k_b, v_w, v_b, gamma):
    x2 = np.asarray(x2, dtype=np.float32)
    x1 = np.asarray(x1, dtype=np.float32)
    g = float(np.asarray(gamma).reshape(-1)[0])
    bf16 = ml_dtypes.bfloat16

    # wblob: per kc, [qkw chunk | g * v_w^T chunk]
    qkw = np.concatenate([np.asarray(q_w).T, np.asarray(k_w).T], axis=1)  # [C,128]
    gvwt = (g * np.asarray(v_w).T).astype(np.float32)  # [C, C]
    wb = np.empty((128, KC * 640), np.float32)
    for kc in range(KC):
        wb[:, 640 * kc : 640 * kc + 128] = qkw[128 * kc : 128 * (kc + 1), :]
        wb[:, 640 * kc + 128 : 640 * (kc + 1)] = gvwt[128 * kc : 128 * (kc + 1), :]
    wb = wb.astype(bf16)
    qkb = (
        np.concatenate([np.asarray(q_b), np.asarray(k_b)])
        .reshape(128, 1)
        .astype(np.float32)
    )

    def pack_x(xfl):
        # [C, S] -> [NG, 128, 2048]: [g, p, j4*512 + kc*128 + sl]
        t = xfl.reshape(KC, 128, NG, 4, 128)  # [kc, p, g, j4, sl]
        return np.ascontiguousarray(
            t.transpose(2, 1, 3, 0, 4).reshape(NG, 128, 2048).astype(bf16)
        )

    in_maps = []
    for b in range(B):
        in_maps.append(
            {
                "xv2": pack_x(x2[b].reshape(C, S)),
                "xv1": pack_x(x1[b].reshape(C, S)),
                "wblob": wb,
                "qkb": qkb,
            }
        )

    # host-side residual (fp32): x + g*v_b
    gvb = (g * np.asarray(v_b)).astype(np.float32)[None, :, None, None]
    return in_maps, (x2 + gvb, x1 + gvb)


def assemble_outputs(res, resid):
    r2, r1 = resid
    y2 = np.empty((B, C, H, W), np.float32)
    y1 = np.empty((B, C, H, W), np.float32)
    for b in range(B):
        yd = np.asarray(res[b]["ydram"])  # [128, NCH*1024] bf16
        # row p = wp*64 + e*32 + m ; col = jp*1024 + t*512 + c
        t = yd.reshape(2, 2, 32, NCH, 2, 512).astype(np.float32)
        # [wp, e, m, jp, t, c] -> y[t][c, h=2m+e, w=2jp+wp]
        t = t.transpose(4, 5, 2, 1, 3, 0)  # [t, c, m, e, jp, wp]
        y2[b] = t[0].reshape(C, H, W)
        y1[b] = t[1].reshape(C, H, W)
    y2 += r2
    y1 += r1
    return y2, y1


def kernel(x2, x1, q_w, q_b, k_w, k_b, v_w, v_b, gamma):
    in_maps, resid = make_in_maps(x2, x1, q_w, q_b, k_w, k_b, v_w, v_b, gamma)
    if not _CACHED:
        _CACHED.append(build_nc())
    nc = _CACHED[0]
    res = run_bass_kernel_spmd(nc, in_maps, list(range(B))).results
    return assemble_outputs(res, resid)


# revision 15
# speedup vs baseline: 2.9480x; 1.2846x over previous
"""Criss-cross attention (CC module) Trainium2 Bass kernel, v6.

Shapes (full): x2,x1 [8, 512, 64, 64] fp32; q_w,k_w [64, 512]; v_w [512, 512];
biases; gamma [1]. Outputs (y2, y1) same shape as x2/x1.

Distribution: data-parallel over batch B=8, one batch element per NeuronCore.

Per-core algorithm (C=512, CQ=64, H=W=64, S=4096), all bf16 matmuls with
fp32 PSUM accumulation:
  Phase B: stream x2,x1 (channel-major, chunk-interleaved); q|k = qkw^T x2
    (+bias via activation); vT chunks = x^T (g*v_w^T) for both tensors,
    written into the resident vp store (h-major rows s=h*64+w, per-chunk
    columns [v2|v1|p2|p1]).
  Phase C: energies as 64x64 blocks; exp() written into block-DIAGONAL att
    tiles (128x128 blocks, zero off-diagonal halves) so each att.v matmul
    runs a full K=128 contraction:
      att1[block t=w//2]: H-attention, rows (wp, H'), cols (wp, h) — both
        natural order, matching the w-major crossing row order (wp, h)
      att2[block j=h//2]: W-attention, rows (hp, W'), cols (hp, w)
  Phase D: per-query normalizer sums via N=1 matmuls in w-major query order
    (att1 block colsums + 2 stride-64 att2 column-gather colsums per
    chunk), reciprocal on DVE -> rw[128,32].
  Phase E (pass 1): part = att2 . v per h-major chunk (one K=128 matmul per
    tensor), PSUM->SBUF on DVE into the vp store; then 2 pure-2D DMA
    scatters (one per hp) write the chunk w-major to DRAM.
  Phase F (pass 2): one contiguous DMA reads each w-major chunk (rows
    (wp, h) natural); psum = ident.p + att1.v; y = psum * rw
    (per-partition scale) -> bf16 -> DRAM (partition-major), 2 chunks per
    write.

All DMAs are pure-2D (<=2 loop levels + contiguous runs) so the DGE spreads
them across all 16 DMA engines; multi-level APs pin to 2 engines.

gamma is folded into the v-weights on the host; the residual x + g*v_b is
added on the host in fp32. exp() needs no max-shift (|E|max ~ 63 << 88).
"""

import numpy as np
import ml_dtypes

import concourse.bass as bass
import concourse.mybir as mybir
import concourse.tile as tile
from concourse import bacc
from concourse.bass_utils import run_bass_kernel_spmd
from concourse.masks import make_identity

BF16 = mybir.dt.bfloat16
F32 = mybir.dt.float32

B, C, H, W = 8, 512, 64, 64
CQ = 64
S = H * W  # 4096
NCH = S // 128  # 32 spatial chunks of 128 rows
KC = C // 128  # 4 contraction chunks
NG = 8  # x stream groups (4 chunks each)

_CACHED = []


def build_nc():
    nc = bacc.Bacc("TRN2", target_bir_lowering=False, debug=False)

    xv2 = nc.dram_tensor("xv2", [NG, 128, 2048], BF16, kind="ExternalInput")
    xv1 = nc.dram_tensor("xv1", [NG, 128, 2048], BF16, kind="ExternalInput")
    wblob = nc.dram_tensor("wblob", [128, KC * 640], BF16, kind="ExternalInput")
    qkb = nc.dram_tensor("qkb", [128, 1], F32, kind="ExternalInput")

    ydram = nc.dram_tensor("ydram", [128, NCH * 1024], BF16, kind="ExternalOutput")

    # DRAM staging for the h-major -> w-major crossing: row w*64+h,
    # cols [v2|v1|p2|p1].
    vp_dram = nc.dram_tensor("vp_dram", [S, 2048], BF16)

    Exp = mybir.ActivationFunctionType.Exp
    Ident = mybir.ActivationFunctionType.Identity

    with tile.TileContext(nc) as tc:
        with (
            tc.tile_pool(name="persist", bufs=1) as pp,
            tc.tile_pool(name="ring", bufs=2) as ring,
            tc.tile_pool(name="psum", bufs=7, space="PSUM") as psp,
        ):
            # ---- persistent tiles ----
            wt = pp.tile([128, KC * 640], BF16, tag="wt", name="wt")
            qkb_t = pp.tile([128, 1], F32, tag="qkb", name="qkb")
            q_t = pp.tile([64, S], BF16, tag="q_t", name="q_t")
            k_t = pp.tile([64, S], BF16, tag="k_t", name="k_t")
            att1 = pp.tile([128, S], BF16, tag="att1", name="att1")
            att2 = pp.tile([128, S], BF16, tag="att2", name="att2")
            # vp store: h-major chunk j cols [2048j : 2048(j+1)] = [v2|v1|p2|p1]
            vp = pp.tile([128, NCH * 2048], BF16, tag="vp", name="vp")
            ident = pp.tile([128, 128], BF16, tag="ident", name="ident")
            ones_col = pp.tile([128, 1], BF16, tag="ones", name="ones")
            rw = pp.tile([128, NCH], F32, tag="rw", name="rw")

            nc.gpsimd.memset(ones_col[:], 1.0)
            nc.gpsimd.memset(att1[:], 0.0)
            nc.gpsimd.memset(att2[:], 0.0)
            make_identity(nc, ident[:])

            nc.sync.dma_start(wt[:], wblob[:, :])
            nc.sync.dma_start(qkb_t[:], qkb[:, :])

            def wq(kc):
                return wt[:, 640 * kc : 640 * kc + 128]

            def wv(kc):
                return wt[:, 640 * kc + 128 : 640 * (kc + 1)]

            # ---- Phase B: stream x; q/k projection + v projection ----
            for g in range(NG):
                xg2 = ring.tile([128, 2048], BF16, tag="x2", bufs=2, name="xg2")
                nc.sync.dma_start(xg2[:], xv2[g, :, :])
                xg1 = ring.tile([128, 2048], BF16, tag="x1", bufs=2, name="xg1")
                nc.sync.dma_start(xg1[:], xv1[g, :, :])

                xg2_v = xg2[:].rearrange("p (j k s) -> p k j s", k=KC, s=128)
                psqk = psp.tile([128, 512], F32, tag="ps", name="ps_qk")
                for kc in range(KC):
                    nc.tensor.matmul(
                        psqk[:],
                        lhsT=wq(kc),
                        rhs=xg2_v[:, kc, :, :],
                        start=(kc == 0),
                        stop=(kc == KC - 1),
                    )
                cols = slice(512 * g, 512 * (g + 1))
                nc.scalar.activation(
                    out=q_t[:, cols], in_=psqk[0:64, :],
                    func=Ident, bias=qkb_t[0:64, 0:1],
                )
                nc.scalar.activation(
                    out=k_t[:, cols], in_=psqk[64:128, :],
                    func=Ident, bias=qkb_t[64:128, 0:1],
                )

                for j4 in range(4):
                    j = 4 * g + j4
                    for t, xg in ((0, xg2), (1, xg1)):
                        psv = psp.tile([128, 512], F32, tag="ps", name="ps_v")
                        for kc in range(KC):
                            nc.tensor.matmul(
                                psv[:],
                                lhsT=xg[
                                    :, 512 * j4 + 128 * kc : 512 * j4 + 128 * (kc + 1)
                                ],
                                rhs=wv(kc),
                                start=(kc == 0),
                                stop=(kc == KC - 1),
                            )
                        nc.vector.tensor_scalar_mul(
                            vp[:, 2048 * j + 512 * t : 2048 * j + 512 * (t + 1)],
                            psv[:],
                            1.0,
                        )

            # natural-order views: s = h*64 + w
            q_hw = q_t[:].rearrange("p (hh w) -> p w hh", w=W)
            k_hw = k_t[:].rearrange("p (hh w) -> p w hh", w=W)
            a2w = att2[:].rearrange("p (hh w) -> p w hh", w=W)

            # ---- Phase C: energies + exp into block-diagonal att tiles ----
            for g8 in range(8):
                psH = psp.tile([64, 512], F32, tag="ps", name="ps_eh")
                psW = psp.tile([64, 512], F32, tag="ps", name="ps_ew")
                for i in range(8):
                    w = 8 * g8 + i
                    nc.tensor.matmul(
                        psH[:, 64 * i : 64 * (i + 1)],
                        lhsT=k_hw[:, w, :],
                        rhs=q_hw[:, w, :],
                        start=True,
                        stop=True,
                    )
                    h = w
                    nc.tensor.matmul(
                        psW[:, 64 * i : 64 * (i + 1)],
                        lhsT=k_t[:, 64 * h : 64 * (h + 1)],
                        rhs=q_t[:, 64 * h : 64 * (h + 1)],
                        start=True,
                        stop=True,
                    )
                psH_v = psH[:].rearrange("p (i2 par q) -> p par i2 q", par=2, q=64)
                psW_v = psW[:].rearrange("p (i2 par q) -> p par i2 q", par=2, q=64)
                for par in range(2):
                    rows = slice(64 * par, 64 * (par + 1))
                    a1v = att1[rows, :].rearrange(
                        "p (t par2 q) -> p t par2 q", par2=2, q=64
                    )
                    a2v = att2[rows, :].rearrange(
                        "p (t par2 q) -> p t par2 q", par2=2, q=64
                    )
                    nc.scalar.activation(
                        out=a1v[:, 4 * g8 : 4 * (g8 + 1), par, :],
                        in_=psH_v[:, par, :, :],
                        func=Exp,
                    )
                    nc.scalar.activation(
                        out=a2v[:, 4 * g8 : 4 * (g8 + 1), par, :],
                        in_=psW_v[:, par, :, :],
                        func=Exp,
                    )

            # ---- Phase D: normalizer sums (w-major natural query order) ----
            psR = psp.tile([128, NCH], F32, tag="psR", bufs=1, name="ps_r")
            for jp in range(NCH):
                nc.tensor.matmul(
                    psR[:, jp : jp + 1],
                    lhsT=att1[:, 128 * jp : 128 * (jp + 1)],
                    rhs=ones_col[:],
                    start=True,
                    stop=False,
                    skip_group_check=True,
                )
                for wp in range(2):
                    nc.tensor.matmul(
                        psR[64 * wp : 64 * (wp + 1), jp : jp + 1],
                        lhsT=a2w[:, 2 * jp + wp, :],
                        rhs=ones_col[:],
                        start=False,
                        stop=(wp == 1),
                        skip_group_check=True,
                        tile_position=(0, 64 * wp),
                    )
            nc.vector.reciprocal(rw[:], psR[:])

            # w-major scatter view of the crossing buffer: row w*64+h
            vpd_w = vp_dram.rearrange("(w hh) qc -> hh w qc", hh=64)

            # ---- Phase E: pass 1 (W-attention, h-major chunks) ----
            for j in range(NCH):
                for t in range(2):
                    pso = psp.tile([128, 512], F32, tag="ps", name="ps_o")
                    nc.tensor.matmul(
                        pso[:],
                        lhsT=att2[:, 128 * j : 128 * (j + 1)],
                        rhs=vp[:, 2048 * j + 512 * t : 2048 * j + 512 * (t + 1)],
                        start=True,
                        stop=True,
                    )
                    nc.vector.tensor_scalar_mul(
                        vp[
                            :,
                            2048 * j + 1024 + 512 * t : 2048 * j + 1024 + 512 * (t + 1),
                        ],
                        pso[:],
                        1.0,
                    )
                for hp in range(2):
                    eng = nc.sync if hp == 0 else nc.scalar
                    eng.dma_start(
                        vpd_w[2 * j + hp, :, :],
                        vp[64 * hp : 64 * (hp + 1), 2048 * j : 2048 * (j + 1)],
                    )

            # ---- Phase F: pass 2 (H-attention, w-major chunks) ----
            for jp in range(NCH):
                vpc = ring.tile([128, 2048], BF16, tag="vpc", bufs=3, name="vpc")
                nc.sync.dma_start(vpc[:], vp_dram[128 * jp : 128 * (jp + 1), :])

                if jp % 2 == 0:
                    yt = ring.tile([128, 2048], BF16, tag="y", bufs=3, name="yt")
                yo = 1024 * (jp % 2)
                for t in range(2):
                    psf = psp.tile([128, 512], F32, tag="ps", name="ps_f")
                    nc.tensor.matmul(
                        psf[:],
                        lhsT=ident[:],
                        rhs=vpc[:, 1024 + 512 * t : 1024 + 512 * (t + 1)],
                        start=True,
                        stop=False,
                        skip_group_check=True,
                    )
                    nc.tensor.matmul(
                        psf[:],
                        lhsT=att1[:, 128 * jp : 128 * (jp + 1)],
                        rhs=vpc[:, 512 * t : 512 * (t + 1)],
                        start=False,
                        stop=True,
                        skip_group_check=True,
                    )
                    nc.scalar.activation(
                        out=yt[:, yo + 512 * t : yo + 512 * (t + 1)],
                        in_=psf[:],
                        func=Ident,
                        scale=rw[:, jp : jp + 1],
                    )
                if jp % 2 == 1:
                    nc.scalar.dma_start(
                        ydram[:, 1024 * (jp - 1) : 1024 * (jp + 1)], yt[:]
                    )

    nc.compile()
    return nc


def make_in_maps(x2, x1, q_w, q_b, k_w, k_b, v_w, v_b, gamma):
    x2 = np.asarray(x2, dtype=np.float32)
    x1 = np.asarray(x1, dtype=np.float32)
    g = float(np.asarray(gamma).reshape(-1)[0])
    bf16 = ml_dtypes.bfloat16

    # wblob: per kc, [qkw chunk | g * v_w^T chunk]
    qkw = np.concatenate([np.asarray(q_w).T, np.asarray(k_w).T], axis=1)  # [C,128]
    gvwt = (g * np.asarray(v_w).T).astype(np.float32)  # [C, C]
    wb = np.empty((128, KC * 640), np.float32)
    for kc in range(KC):
        wb[:, 640 * kc : 640 * kc + 128] = qkw[128 * kc : 128 * (kc + 1), :]
        wb[:, 640 * kc + 128 : 640 * (kc + 1)] = gvwt[128 * kc : 128 * (kc + 1), :]
    wb = wb.astype(bf16)
    qkb = (
        np.concatenate([np.asarray(q_b), np.asarray(k_b)])
        .reshape(128, 1)
        .astype(np.float32)
    )

    def pack_x(xfl):
        # [C, S] -> [NG, 128, 2048]: [g, p, j4*512 + kc*128 + sl]
        t = xfl.reshape(KC, 128, NG, 4, 128)  # [kc, p, g, j4, sl]
        return np.ascontiguousarray(
            t.transpose(2, 1, 3, 0, 4).reshape(NG, 128, 2048).astype(bf16)
        )

    in_maps = []
    for b in range(B):
        in_maps.append(
            {
                "xv2": pack_x(x2[b].reshape(C, S)),
                "xv1": pack_x(x1[b].reshape(C, S)),
                "wblob": wb,
                "qkb": qkb,
            }
        )

    # host-side residual (fp32): x + g*v_b
    gvb = (g * np.asarray(v_b)).astype(np.float32)[None, :, None, None]
    return in_maps, (x2 + gvb, x1 + gvb)


def assemble_outputs(res, resid):
    r2, r1 = resid
    y2 = np.empty((B, C, H, W), np.float32)
    y1 = np.empty((B, C, H, W), np.float32)
    for b in range(B):
        yd = np.asarray(res[b]["ydram"])  # [128, NCH*1024] bf16
        # row p = wp*64 + h ; col = jp*1024 + t*512 + c ; w = 2jp+wp
        t = yd.reshape(2, 64, NCH, 2, 512).astype(np.float32)
        # [wp, h, jp, t, c] -> y[t][c, h, w=2jp+wp]
        t = t.transpose(3, 4, 1, 2, 0)  # [t, c, h, jp, wp]
        y2[b] = t[0].reshape(C, H, W)
        y1[b] = t[1].reshape(C, H, W)
    y2 += r2
    y1 += r1
    return y2, y1


def kernel(x2, x1, q_w, q_b, k_w, k_b, v_w, v_b, gamma):
    in_maps, resid = make_in_maps(x2, x1, q_w, q_b, k_w, k_b, v_w, v_b, gamma)
    if not _CACHED:
        _CACHED.append(build_nc())
    nc = _CACHED[0]
    res = run_bass_kernel_spmd(nc, in_maps, list(range(B))).results
    return assemble_outputs(res, resid)


# revision 17
# speedup vs baseline: 3.2673x; 1.1083x over previous
"""Criss-cross attention (CC module) Trainium2 Bass kernel, v6.

Shapes (full): x2,x1 [8, 512, 64, 64] fp32; q_w,k_w [64, 512]; v_w [512, 512];
biases; gamma [1]. Outputs (y2, y1) same shape as x2/x1.

Distribution: data-parallel over batch B=8, one batch element per NeuronCore.

Per-core algorithm (C=512, CQ=64, H=W=64, S=4096), all bf16 matmuls with
fp32 PSUM accumulation:
  Phase B: stream x2,x1 (channel-major, chunk-interleaved); q|k = qkw^T x2
    (+bias via activation); vT chunks = x^T (g*v_w^T) for both tensors,
    written into the resident vp store (h-major rows s=h*64+w, per-chunk
    columns [v2|v1|p2|p1]).
  Phase C: energies as 64x64 blocks; exp() written into block-DIAGONAL att
    tiles (128x128 blocks, zero off-diagonal halves) so each att.v matmul
    runs a full K=128 contraction:
      att1[block t=w//2]: H-attention, rows (wp, H'), cols (wp, h) — both
        natural order, matching the w-major crossing row order (wp, h)
      att2[block j=h//2]: W-attention, rows (hp, W'), cols (hp, w)
  Phase D: per-query normalizer sums via N=1 matmuls in w-major query order
    (att1 block colsums + 2 stride-64 att2 column-gather colsums per
    chunk), reciprocal on DVE -> rw[128,32].
  Phase E (pass 1): part = att2 . v per h-major chunk (one K=128 matmul per
    tensor), PSUM->SBUF on DVE into the vp store; then 2 pure-2D DMA
    scatters (one per hp) write the chunk w-major to DRAM.
  Phase F (pass 2): one contiguous DMA reads each w-major chunk (rows
    (wp, h) natural); psum = ident.p + att1.v; y = psum * rw
    (per-partition scale) -> bf16 -> DRAM (partition-major), 2 chunks per
    write.

All DMAs are pure-2D (<=2 loop levels + contiguous runs) so the DGE spreads
them across all 16 DMA engines; multi-level APs pin to 2 engines.

gamma is folded into the v-weights on the host; the residual x + g*v_b is
added on the host in fp32. exp() needs no max-shift (|E|max ~ 63 << 88).
"""

import numpy as np
import ml_dtypes

import concourse.bass as bass
import concourse.mybir as mybir
import concourse.tile as tile
from concourse import bacc
from concourse.bass_utils import run_bass_kernel_spmd
from concourse.masks import make_identity

BF16 = mybir.dt.bfloat16
FP8 = mybir.dt.float8e4
F32 = mybir.dt.float32

B, C, H, W = 8, 512, 64, 64
CQ = 64
S = H * W  # 4096
NCH = S // 128  # 32 spatial chunks of 128 rows
KC = C // 128  # 4 contraction chunks
NG = 8  # x stream groups (4 chunks each)

_CACHED = []


def build_nc():
    nc = bacc.Bacc("TRN2", target_bir_lowering=False, debug=False)

    xv2 = nc.dram_tensor("xv2", [NG, 128, 2048], BF16, kind="ExternalInput")
    xv1 = nc.dram_tensor("xv1", [NG, 128, 2048], BF16, kind="ExternalInput")
    wblob = nc.dram_tensor("wblob", [128, KC * 640], BF16, kind="ExternalInput")
    qkb = nc.dram_tensor("qkb", [128, 1], F32, kind="ExternalInput")

    ydram = nc.dram_tensor("ydram", [128, NCH * 1024], BF16, kind="ExternalOutput")

    # DRAM staging for the h-major -> w-major crossing: row w*64+h,
    # cols [v2|v1|p2|p1].
    vp_dram = nc.dram_tensor("vp_dram", [S, 2048], FP8)

    Exp = mybir.ActivationFunctionType.Exp
    Ident = mybir.ActivationFunctionType.Identity

    with tile.TileContext(nc) as tc:
        with (
            tc.tile_pool(name="persist", bufs=1) as pp,
            tc.tile_pool(name="ring", bufs=2) as ring,
            tc.tile_pool(name="psum", bufs=6, space="PSUM") as psp,
        ):
            # ---- persistent tiles ----
            wt = pp.tile([128, KC * 640], BF16, tag="wt", name="wt")
            qkb_t = pp.tile([128, 1], F32, tag="qkb", name="qkb")
            q_t = pp.tile([64, S], BF16, tag="q_t", name="q_t")
            k_t = pp.tile([64, S], BF16, tag="k_t", name="k_t")
            att1 = pp.tile([128, S], BF16, tag="att1", name="att1")
            att2 = pp.tile([128, S], BF16, tag="att2", name="att2")
            # vp store: h-major chunk j cols [2048j : 2048(j+1)] = [v2|v1|p2|p1]
            vp = pp.tile([128, NCH * 2048], FP8, tag="vp", name="vp")
            ones_col = pp.tile([128, 1], BF16, tag="ones", name="ones")
            rw = pp.tile([128, NCH], F32, tag="rw", name="rw")
            rh = pp.tile([128, NCH], F32, tag="rh", name="rh")

            nc.gpsimd.memset(ones_col[:], 1.0)
            nc.gpsimd.memset(att1[:], 0.0)
            nc.gpsimd.memset(att2[:], 0.0)

            nc.sync.dma_start(wt[:], wblob[:, :])
            nc.sync.dma_start(qkb_t[:], qkb[:, :])

            def wq(kc):
                return wt[:, 640 * kc : 640 * kc + 128]

            def wv(kc):
                return wt[:, 640 * kc + 128 : 640 * (kc + 1)]

            # ---- Phase B: stream x; q/k projection + v projection ----
            for g in range(NG):
                xg2 = ring.tile([128, 2048], BF16, tag="x2", bufs=2, name="xg2")
                nc.sync.dma_start(xg2[:], xv2[g, :, :])
                xg1 = ring.tile([128, 2048], BF16, tag="x1", bufs=2, name="xg1")
                nc.sync.dma_start(xg1[:], xv1[g, :, :])

                xg2_v = xg2[:].rearrange("p (j k s) -> p k j s", k=KC, s=128)
                psqk = psp.tile([128, 512], F32, tag="ps", name="ps_qk")
                for kc in range(KC):
                    nc.tensor.matmul(
                        psqk[:],
                        lhsT=wq(kc),
                        rhs=xg2_v[:, kc, :, :],
                        start=(kc == 0),
                        stop=(kc == KC - 1),
                    )
                cols = slice(512 * g, 512 * (g + 1))
                nc.scalar.activation(
                    out=q_t[:, cols], in_=psqk[0:64, :],
                    func=Ident, bias=qkb_t[0:64, 0:1],
                )
                nc.scalar.activation(
                    out=k_t[:, cols], in_=psqk[64:128, :],
                    func=Ident, bias=qkb_t[64:128, 0:1],
                )

                for j4 in range(4):
                    j = 4 * g + j4
                    for t, xg in ((0, xg2), (1, xg1)):
                        psv = psp.tile([128, 512], F32, tag="ps", name="ps_v")
                        for kc in range(KC):
                            nc.tensor.matmul(
                                psv[:],
                                lhsT=xg[
                                    :, 512 * j4 + 128 * kc : 512 * j4 + 128 * (kc + 1)
                                ],
                                rhs=wv(kc),
                                start=(kc == 0),
                                stop=(kc == KC - 1),
                            )
                        nc.vector.tensor_scalar_mul(
                            vp[:, 2048 * j + 512 * t : 2048 * j + 512 * (t + 1)],
                            psv[:],
                            1.0,
                        )

            # natural-order views: s = h*64 + w
            q_hw = q_t[:].rearrange("p (hh w) -> p w hh", w=W)
            k_hw = k_t[:].rearrange("p (hh w) -> p w hh", w=W)
            a2w = att2[:].rearrange("p (hh w) -> p w hh", w=W)

            # ---- Phase C: energies + exp into block-diagonal att tiles ----
            for g8 in range(8):
                psH = psp.tile([64, 512], F32, tag="ps", name="ps_eh")
                psW = psp.tile([64, 512], F32, tag="ps", name="ps_ew")
                for i in range(8):
                    w = 8 * g8 + i
                    nc.tensor.matmul(
                        psH[:, 64 * i : 64 * (i + 1)],
                        lhsT=k_hw[:, w, :],
                        rhs=q_hw[:, w, :],
                        start=True,
                        stop=True,
                    )
                    h = w
                    nc.tensor.matmul(
                        psW[:, 64 * i : 64 * (i + 1)],
                        lhsT=k_t[:, 64 * h : 64 * (h + 1)],
                        rhs=q_t[:, 64 * h : 64 * (h + 1)],
                        start=True,
                        stop=True,
                    )
                psH_v = psH[:].rearrange("p (i2 par q) -> p par i2 q", par=2, q=64)
                psW_v = psW[:].rearrange("p (i2 par q) -> p par i2 q", par=2, q=64)
                for par in range(2):
                    rows = slice(64 * par, 64 * (par + 1))
                    a1v = att1[rows, :].rearrange(
                        "p (t par2 q) -> p t par2 q", par2=2, q=64
                    )
                    a2v = att2[rows, :].rearrange(
                        "p (t par2 q) -> p t par2 q", par2=2, q=64
                    )
                    nc.scalar.activation(
                        out=a1v[:, 4 * g8 : 4 * (g8 + 1), par, :],
                        in_=psH_v[:, par, :, :],
                        func=Exp,
                    )
                    nc.scalar.activation(
                        out=a2v[:, 4 * g8 : 4 * (g8 + 1), par, :],
                        in_=psW_v[:, par, :, :],
                        func=Exp,
                    )

            # ---- Phase D: normalizer sums (w-major natural query order) ----
            psR = psp.tile([128, NCH], F32, tag="psR", bufs=1, name="ps_r")
            for jp in range(NCH):
                nc.tensor.matmul(
                    psR[:, jp : jp + 1],
                    lhsT=att1[:, 128 * jp : 128 * (jp + 1)],
                    rhs=ones_col[:],
                    start=True,
                    stop=False,
                    skip_group_check=True,
                )
                for wp in range(2):
                    nc.tensor.matmul(
                        psR[64 * wp : 64 * (wp + 1), jp : jp + 1],
                        lhsT=a2w[:, 2 * jp + wp, :],
                        rhs=ones_col[:],
                        start=False,
                        stop=(wp == 1),
                        skip_group_check=True,
                        tile_position=(0, 64 * wp),
                    )
            nc.vector.reciprocal(rw[:], psR[:])

            # h-major normalizer: r_h[p=(hp,w), j] for pass-1 part scaling
            a1h = att1[:].rearrange("p (w hh) -> p hh w", hh=64)
            psRh = psp.tile([128, NCH], F32, tag="psRh", bufs=1, name="ps_rh")
            for j in range(NCH):
                nc.tensor.matmul(
                    psRh[:, j : j + 1],
                    lhsT=att2[:, 128 * j : 128 * (j + 1)],
                    rhs=ones_col[:],
                    start=True,
                    stop=False,
                    skip_group_check=True,
                )
                for hp in range(2):
                    nc.tensor.matmul(
                        psRh[64 * hp : 64 * (hp + 1), j : j + 1],
                        lhsT=a1h[:, 2 * j + hp, :],
                        rhs=ones_col[:],
                        start=False,
                        stop=(hp == 1),
                        skip_group_check=True,
                        tile_position=(0, 64 * hp),
                    )
            nc.vector.reciprocal(rh[:], psRh[:])

            # w-major scatter view of the crossing buffer: row w*64+h
            vpd_w = vp_dram.rearrange("(w hh) qc -> hh w qc", hh=64)

            # ---- Phase E: pass 1 (W-attention, h-major chunks) ----
            for j in range(NCH):
                for t in range(2):
                    pso = psp.tile([128, 512], F32, tag="ps", name="ps_o")
                    nc.tensor.matmul(
                        pso[:],
                        lhsT=att2[:, 128 * j : 128 * (j + 1)],
                        rhs=vp[:, 2048 * j + 512 * t : 2048 * j + 512 * (t + 1)],
                        start=True,
                        stop=True,
                    )
                    nc.vector.tensor_scalar_mul(
                        vp[
                            :,
                            2048 * j + 1024 + 512 * t : 2048 * j + 1024 + 512 * (t + 1),
                        ],
                        pso[:],
                        rh[:, j : j + 1],
                    )
                for hp in range(2):
                    eng = nc.sync if hp == 0 else nc.scalar
                    eng.dma_start(
                        vpd_w[2 * j + hp, :, :],
                        vp[64 * hp : 64 * (hp + 1), 2048 * j : 2048 * (j + 1)],
                    )

            # ---- Phase F: pass 2 (H-attention, w-major chunks) ----
            for jp in range(NCH):
                vpc = ring.tile([128, 2048], FP8, tag="vpc", bufs=4, name="vpc")
                nc.sync.dma_start(vpc[:], vp_dram[128 * jp : 128 * (jp + 1), :])

                if jp % 2 == 0:
                    yt = ring.tile([128, 2048], BF16, tag="y", bufs=3, name="yt")
                yo = 1024 * (jp % 2)
                for t in range(2):
                    psf = psp.tile([128, 512], F32, tag="ps", name="ps_f")
                    nc.tensor.matmul(
                        psf[:],
                        lhsT=att1[:, 128 * jp : 128 * (jp + 1)],
                        rhs=vpc[:, 512 * t : 512 * (t + 1)],
                        start=True,
                        stop=True,
                    )
                    nc.vector.scalar_tensor_tensor(
                        out=yt[:, yo + 512 * t : yo + 512 * (t + 1)],
                        in0=psf[:],
                        scalar=rw[:, jp : jp + 1],
                        in1=vpc[:, 1024 + 512 * t : 1024 + 512 * (t + 1)],
                        op0=mybir.AluOpType.mult,
                        op1=mybir.AluOpType.add,
                    )
                if jp % 2 == 1:
                    nc.scalar.dma_start(
                        ydram[:, 1024 * (jp - 1) : 1024 * (jp + 1)], yt[:]
                    )

    nc.compile()
    return nc


def make_in_maps(x2, x1, q_w, q_b, k_w, k_b, v_w, v_b, gamma):
    x2 = np.asarray(x2, dtype=np.float32)
    x1 = np.asarray(x1, dtype=np.float32)
    g = float(np.asarray(gamma).reshape(-1)[0])
    bf16 = ml_dtypes.bfloat16

    # wblob: per kc, [qkw chunk | g * v_w^T chunk]
    qkw = np.concatenate([np.asarray(q_w).T, np.asarray(k_w).T], axis=1)  # [C,128]
    gvwt = (g * np.asarray(v_w).T).astype(np.float32)  # [C, C]
    wb = np.empty((128, KC * 640), np.float32)
    for kc in range(KC):
        wb[:, 640 * kc : 640 * kc + 128] = qkw[128 * kc : 128 * (kc + 1), :]
        wb[:, 640 * kc + 128 : 640 * (kc + 1)] = gvwt[128 * kc : 128 * (kc + 1), :]
    wb = wb.astype(bf16)
    qkb = (
        np.concatenate([np.asarray(q_b), np.asarray(k_b)])
        .reshape(128, 1)
        .astype(np.float32)
    )

    def pack_x(xfl):
        # [C, S] -> [NG, 128, 2048]: [g, p, j4*512 + kc*128 + sl]
        t = xfl.reshape(KC, 128, NG, 4, 128)  # [kc, p, g, j4, sl]
        return np.ascontiguousarray(
            t.transpose(2, 1, 3, 0, 4).reshape(NG, 128, 2048).astype(bf16)
        )

    in_maps = []
    for b in range(B):
        in_maps.append(
            {
                "xv2": pack_x(x2[b].reshape(C, S)),
                "xv1": pack_x(x1[b].reshape(C, S)),
                "wblob": wb,
                "qkb": qkb,
            }
        )

    # host-side residual (fp32): x + g*v_b
    gvb = (g * np.asarray(v_b)).astype(np.float32)[None, :, None, None]
    return in_maps, (x2 + gvb, x1 + gvb)


def assemble_outputs(res, resid):
    r2, r1 = resid
    y2 = np.empty((B, C, H, W), np.float32)
    y1 = np.empty((B, C, H, W), np.float32)
    for b in range(B):
        yd = np.asarray(res[b]["ydram"])  # [128, NCH*1024] bf16
        # row p = wp*64 + h ; col = jp*1024 + t*512 + c ; w = 2jp+wp
        t = yd.reshape(2, 64, NCH, 2, 512).astype(np.float32)
        # [wp, h, jp, t, c] -> y[t][c, h, w=2jp+wp]
        t = t.transpose(3, 4, 1, 2, 0)  # [t, c, h, jp, wp]
        y2[b] = t[0].reshape(C, H, W)
        y1[b] = t[1].reshape(C, H, W)
    y2 += r2
    y1 += r1
    return y2, y1


def kernel(x2, x1, q_w, q_b, k_w, k_b, v_w, v_b, gamma):
    in_maps, resid = make_in_maps(x2, x1, q_w, q_b, k_w, k_b, v_w, v_b, gamma)
    if not _CACHED:
        _CACHED.append(build_nc())
    nc = _CACHED[0]
    res = run_bass_kernel_spmd(nc, in_maps, list(range(B))).results
    return assemble_outputs(res, resid)
